# revision 25
# baseline (speedup 1.0000x reference)
"""Differential-Transformer attention (DiffAttn) Trainium2 Bass kernel.

Sharding: 8 cores = 2 (batch) x 4 (head-group tensor parallel).
Core c = 4*b + t handles batch b, query heads 4t..4t+3, kv head t,
and the two v-heads its query heads need (t//2 and t//2+2).
o_proj is row-parallel: each core returns a partial [L, HID] product;
the host sums the 4 partials per batch (the "unshard" step).

The datapath runs in fp16 (11-bit mantissa: same accuracy class as
the PE's fp32r mode, measured mean rel err ~5e-3 for either), which
halves DMA bytes and SBUF footprint, runs matmuls at full PE speed,
and -- unlike fp32r -- is a native dtype, so DMA can feed matmuls
directly with no on-chip rounding passes.  All value magnitudes here
are < 1e3 and weights are ~0.02-scale, far from fp16 range limits;
the unnormalized-softmax exp gets a constant -2 bias for overflow
headroom, which cancels exactly in the RMS-norm fold below.

Structure: ONE fused pipeline over the four 512-wide q-slices:
projections+RoPE (PE/DVE-heavy, no ACT) for slice j run right before
attention for slice j (ACT-heavy exp), and o_proj tiles for slice
j-1 are interleaved into attention j's score/AV pair loop to keep PE
fed where it would otherwise wait on ACT's exp.  Each head's
normalization (Ln/Exp/scale) is deferred into the next head's stream
so the ACT queue never blocks the next head's first exp.

Softmax runs without max-subtraction (score magnitudes are bounded
~5) on transposed score tiles S^T[k, q] so the AV matmul needs no
transposes.  Score/AV matmuls and the exp are trimmed to the causal
triangle at 128-block granularity (256-row floor on moving dims,
below which the PE runs at 1/4 throughput anyway).  RoPE: the sin
table is sign-folded on the host (rows 0..63 negated), so
q' = q*cos + swap(q)*sin_folded needs three muls and one add, with
the partition swap read from PSUM (PSUM+SBUF input pairs are exempt
from the same-base-partition rule).  The softmax denominator and the
RMS-norm rsqrt fold into one scale from an all-ones stationary
matmul (sum of squares replicated across partitions):
  comb = exp(-0.5*ln(ssq/128))  (= rsqrt(mean u^2); the softmax 1/den
and the exp bias cancel exactly inside the RMS norm, and eps*den^2
is negligible).  The RMSNorm weight and the (1-lambda_init) factor
are folded into Wo on the host.
"""

import os
import sys

import numpy as np

for _p in ("/opt/trn_rl_repo",):
    if _p not in sys.path and os.path.isdir(_p):
        sys.path.insert(0, _p)

B = 2
L = 2048
HID = 2048
D = 128
H = 16
NH = 4            # query heads per core
CT = HID // 128   # contraction tiles for the projections
EPS = 1e-6
LAMBDA_INIT = 0.2
EXP_BIAS = -2.0   # overflow headroom for fp16 exp; cancels in the norm

_CACHE = {}


def _build(length=L):
    from concourse import bacc
    import concourse.mybir as mybir
    import concourse.tile as tile

    f32 = mybir.dt.float32
    fp16 = mybir.dt.float16
    Act = mybir.ActivationFunctionType

    # Both ACT functions used here (Exp, Ln) live together in the
    # 'natural_log_exp_and_others' LUT set, but the table-load pass assigns
    # each activation the first set containing its function, which alternates
    # sets and inserts a ~1.3us table reload per switch.  Restrict the pass's
    # view to that one set -> exactly one load total.
    _orig_tables = bacc.get_activation_tables

    def _only_ln_exp(arch):
        t = _orig_tables(arch)
        keep = "natural_log_exp_and_others"
        if keep not in t:
            return t
        return {name: (s if name == keep else set()) for name, s in t.items()}

    bacc.get_activation_tables = _only_ln_exp
    try:
        return _build_inner(length, bacc, mybir, tile, f32, fp16, Act)
    finally:
        bacc.get_activation_tables = _orig_tables


def _build_inner(length, bacc, mybir, tile, f32, fp16, Act):
    NJ = length // 512    # q-slices
    NLB = length // 128   # l/k blocks

    nc = bacc.Bacc()
    hsT = nc.dram_tensor("hsT", [HID, length], fp16, kind="ExternalInput")
    cs = nc.dram_tensor("cs", [D, 2, length], fp16, kind="ExternalInput")
    wqkv = nc.dram_tensor("wqkv", [HID, 7 * D], fp16, kind="ExternalInput")
    wo = nc.dram_tensor("wo", [NH * D, HID], fp16, kind="ExternalInput")
    lam = nc.dram_tensor("lam", [D, 1], f32, kind="ExternalInput")
    masks = nc.dram_tensor("masks", [D, 512], fp16, kind="ExternalInput")
    part = nc.dram_tensor("part", [length, HID], fp16, kind="ExternalOutput")

    inv_sqrt_d = 1.0 / np.sqrt(np.float32(D))
    # per-block start column of the trimmed score/AV region (within the
    # 512-wide q slice), floored at 256 moving rows
    trim0 = (0, 128, 256, 256)

    with tile.TileContext(nc) as tc:
        with tc.tile_pool(name="persist", bufs=1) as persist, \
             tc.tile_pool(name="qTp", bufs=2) as qTp, \
             tc.tile_pool(name="finp", bufs=2) as finp, \
             tc.tile_pool(name="hsp", bufs=6) as hsp, \
             tc.tile_pool(name="csp", bufs=2) as csp, \
             tc.tile_pool(name="btmp", bufs=2) as btmp, \
             tc.tile_pool(name="sep", bufs=3) as sep, \
             tc.tile_pool(name="ufp", bufs=2) as ufp, \
             tc.tile_pool(name="ctmp", bufs=2) as ctmp, \
             tc.tile_pool(name="dout", bufs=4) as dout, \
             tc.tile_pool(name="mm_p", bufs=2, space="PSUM") as mm_p, \
             tc.tile_pool(name="pss_p", bufs=2, space="PSUM") as pss_p, \
             tc.tile_pool(name="pso_p", bufs=2, space="PSUM") as pso_p:

            kT = persist.tile([D, length], fp16, tag="kT")
            veff = persist.tile([D, NLB, D], fp16, tag="veff")
            wqkv_t = persist.tile([D, CT, 7 * D], fp16, tag="wqkv")
            wo_t = persist.tile([D, NH, HID], fp16, tag="wo")
            mask_t = persist.tile([D, 512], fp16, tag="mask")
            ebias_t = persist.tile([D, 1], f32, tag="ebias")
            lam_t = persist.tile([D, 1], f32, tag="lam")
            ones_t = persist.tile([D, D], fp16, tag="ones")

            # ---------------- preamble loads ------------------------------
            nc.sync.dma_start(out=lam_t, in_=lam[:, :])
            nc.sync.dma_start(out=mask_t, in_=masks[:, :])
            nc.vector.memset(ones_t, 1.0)
            nc.vector.memset(ebias_t, EXP_BIAS)

            wqkv_r = wqkv.rearrange("(c p) m -> p c m", p=D)
            wo_r = wo.rearrange("(h p) m -> p h m", p=D)
            hs_r = hsT.rearrange("(c p) m -> p c m", p=D)

            def load_hs_slice(j):
                tiles = []
                for g in range(4):
                    t = hsp.tile([D, 4, 512], fp16, tag="hs",
                                 name=f"hs_{j}_{g}")
                    nc.sync.dma_start(
                        out=t, in_=hs_r[:, 4 * g:4 * (g + 1),
                                        512 * j:512 * (j + 1)])
                    for i in range(4):
                        tiles.append(t[:, i, :])
                return tiles

            def load_cs(j):
                t = csp.tile([D, 2, 512], fp16, tag="cs", name=f"cs_{j}")
                nc.sync.dma_start(out=t,
                                  in_=cs[:, :, 512 * j:512 * (j + 1)])
                return t

            hs_tiles = {}
            cs_tiles = {}
            # startup: weight loads issue from the scalar engine's DGE queue
            # while hs loads issue from sync -- the two queues' transfers
            # interleave on the DMA engines, roughly doubling the effective
            # supply rate during the DMA-bound first slice
            for g in range(4):
                gc = slice(4 * g, 4 * (g + 1))
                nc.scalar.dma_start(out=wqkv_t[:, gc, :], in_=wqkv_r[:, gc, :])
            hs_tiles[0] = load_hs_slice(0)
            cs_tiles[0] = load_cs(0)
            cs_tiles[1] = load_cs(1)
            hs_tiles[1] = load_hs_slice(1)
            for h in range(NH):
                nc.scalar.dma_start(out=wo_t[:, h, :], in_=wo_r[:, h, :])

            pending_oproj = []   # emit-closures for o_proj tiles
            pending_norm = [None]
            pair_ctr = [0]

            def emit_norm():
                if pending_norm[0] is not None:
                    pending_norm[0]()
                    pending_norm[0] = None

            def emit_oproj(n=1):
                for _ in range(min(n, len(pending_oproj))):
                    pending_oproj.pop(0)()

            fin_tiles = {}

            for j in range(NJ):
                sl = slice(512 * j, 512 * (j + 1))
                # prefetch next cs and the hs slice after next
                if j + 2 < NJ:
                    cs_tiles[j + 2] = load_cs(j + 2)
                    hs_tiles[j + 2] = load_hs_slice(j + 2)
                chunks = hs_tiles.pop(j)
                cs_s = cs_tiles.pop(j)

                # ---------------- projections + RoPE ----------------------
                qTj = qTp.tile([D, NH, 512], fp16, tag="qT", name=f"qT_{j}")

                def rope(ps, db, qTj=qTj, cs_s=cs_s, sl=sl):
                    t1 = btmp.tile([D, 512], fp16, tag="t1")
                    t2 = btmp.tile([D, 512], fp16, tag="t2")
                    nc.vector.tensor_mul(t1, ps, cs_s[:, 0, :])
                    nc.vector.tensor_mul(t2[0:64, :], ps[64:128, :],
                                         cs_s[0:64, 1, :])
                    nc.vector.tensor_mul(t2[64:128, :], ps[0:64, :],
                                         cs_s[64:128, 1, :])
                    dst = qTj[:, db, :] if db < NH else kT[:, sl]
                    nc.vector.tensor_add(dst, t1, t2)

                def vcomb(psv256, i, j=j):
                    # veff = v_lo + (-lam)*v_hi (lam negated on the host)
                    tv = btmp.tile([D, D], f32, tag="tv")
                    nc.vector.tensor_scalar_mul(tv, psv256[:, 128:256], lam_t)
                    nc.vector.tensor_add(veff[:, 4 * j + i, :],
                                         psv256[:, 0:128], tv)

                if j == 0:
                    # slice 0 is paced by the input DMA: walk chunk-major so
                    # the PE does ALL of each chunk's work (5 projection rows
                    # + 4 v accumulations) the moment it lands, using the
                    # still-idle attention PSUM banks as extra accumulators
                    psA = pss_p.tile([D, 2, 512], f32, tag="s", name="p0a")
                    psB = pss_p.tile([D, 2, 512], f32, tag="s", name="p0b")
                    psC = pso_p.tile([D, 512], f32, tag="o", name="p0c")
                    accs = [psA[:, 0, :], psA[:, 1, :], psB[:, 0, :],
                            psB[:, 1, :], psC]
                    for c in range(CT - 1):
                        for db in range(NH + 1):
                            nc.tensor.matmul(
                                accs[db],
                                wqkv_t[:, c, 128 * db:128 * (db + 1)],
                                chunks[c], start=(c == 0), stop=False,
                                skip_group_check=True)
                    # last chunk per-db, k first, each RoPE drain overlapped
                    # with a v accumulation chain on the PE
                    c = CT - 1
                    vq = []
                    for n, db in enumerate((NH, 0, 1, 2, 3)):
                        nc.tensor.matmul(
                            accs[db], wqkv_t[:, c, 128 * db:128 * (db + 1)],
                            chunks[c], start=False, stop=True,
                            skip_group_check=True)
                        rope(accs[db], db)
                        if n >= 1:
                            i = n - 1
                            psv = mm_p.tile([D, 512], f32, tag="mm")
                            for cc in range(CT):
                                nc.tensor.matmul(
                                    psv[:, 0:256],
                                    chunks[cc][:, 128 * i:128 * (i + 1)],
                                    wqkv_t[:, cc, 640:896],
                                    start=(cc == 0), stop=(cc == CT - 1))
                            vq.append((psv, i))
                    for psv, i in vq:
                        vcomb(psv[:, 0:512], i)
                else:
                    for db in range(NH + 1):
                        ps = mm_p.tile([D, 512], f32, tag="mm")
                        for c in range(CT):
                            nc.tensor.matmul(
                                ps, wqkv_t[:, c, 128 * db:128 * (db + 1)],
                                chunks[c],
                                start=(c == 0), stop=(c == CT - 1))
                        rope(ps, db)
                    for i in range(4):
                        psv = mm_p.tile([D, 512], f32, tag="mm")
                        for c in range(CT):
                            nc.tensor.matmul(
                                psv[:, 0:256],
                                chunks[c][:, 128 * i:128 * (i + 1)],
                                wqkv_t[:, c, 640:896],
                                start=(c == 0), stop=(c == CT - 1))
                        vcomb(psv[:, 0:512], i)

                # ---------------- attention -------------------------------
                finj = finp.tile([D, NH, 512], fp16, tag="fin",
                                 name=f"fin_{j}")
                fin_tiles[j] = finj
                nkb = 4 * j + 4
                npair = nkb // 2
                # pace pending o_proj pops across ALL this slice's pair
                # iterations so the last heads aren't starved of PE filler
                slots = NH * max(npair - 1, 1)
                pop_frac = [0.0]
                pop_rate = len(pending_oproj) / slots if slots else 0.0
                for h in range(NH):
                    pso = pso_p.tile([D, 512], f32, tag="o")
                    se_l = {}

                    def score_pair(t, j=j, h=h):
                        pss = pss_p.tile([D, 2, 512], f32, tag="s")
                        se = sep.tile([D, 2, 512], fp16, tag="se")
                        ib0 = 2 * t - 4 * j
                        if ib0 == 2:
                            # second diagonal pair: both blocks cover q
                            # [256:512] -- pack block ib=2's scores into
                            # columns [0:256] and ib=3's into [256:512] of
                            # ONE psum bank, so a single contiguous exp and
                            # a single [tri|ones|zero|tri] mask cover both
                            nc.tensor.matmul(
                                pss[:, 0, 0:256],
                                kT[:, 128 * (4 * j + 2):128 * (4 * j + 3)],
                                qTj[:, h, 256:512], start=True, stop=True)
                            nc.tensor.matmul(
                                pss[:, 0, 256:512],
                                kT[:, 128 * (4 * j + 3):128 * (4 * j + 4)],
                                qTj[:, h, 256:512], start=True, stop=True)
                            nc.scalar.activation(
                                se[:, 0, :], pss[:, 0, :], Act.Exp,
                                bias=ebias_t, scale=float(inv_sqrt_d))
                            nc.vector.tensor_mul(se[:, 0, :], se[:, 0, :],
                                                 mask_t)
                        else:
                            for i in (0, 1):
                                kb = 2 * t + i
                                ib = kb - 4 * j   # in-slice block index
                                r0 = trim0[ib] if ib >= 0 else 0
                                nc.tensor.matmul(
                                    pss[:, i, r0:512],
                                    kT[:, 128 * kb:128 * (kb + 1)],
                                    qTj[:, h, r0:512],
                                    start=True, stop=True)
                                if ib < 0 and i == 1:
                                    # non-diagonal pair: one exp for both
                                    nc.scalar.activation(
                                        se, pss, Act.Exp,
                                        bias=ebias_t, scale=float(inv_sqrt_d))
                                elif ib >= 0:
                                    nc.scalar.activation(
                                        se[:, i, r0:512], pss[:, i, r0:512],
                                        Act.Exp,
                                        bias=ebias_t, scale=float(inv_sqrt_d))
                                    # mask the diagonal 128-block
                                    m0 = 128 * ib
                                    nc.vector.tensor_mul(
                                        se[:, i, m0:m0 + 128],
                                        se[:, i, m0:m0 + 128],
                                        mask_t[:, 0:128])
                        se_l[t] = se

                    score_pair(0)
                    for t in range(npair):
                        if t + 1 < npair:
                            score_pair(t + 1)
                            if t == 0:
                                # previous head's normalization: emitted two
                                # exps deep so its Ln (waiting on the ssq
                                # matmul) never stalls the ACT queue head
                                emit_norm()
                            pop_frac[0] += pop_rate
                            if pop_frac[0] >= 1.0:
                                pop_frac[0] -= 1.0
                                emit_oproj(1)
                        se = se_l.pop(t)
                        if 2 * t - 4 * j == 2:
                            nc.tensor.matmul(pso[:, 256:512],
                                             veff[:, 4 * j + 2, :],
                                             se[:, 0, 0:256],
                                             start=False, stop=False)
                            nc.tensor.matmul(pso[:, 256:512],
                                             veff[:, 4 * j + 3, :],
                                             se[:, 0, 256:512],
                                             start=False, stop=True)
                        else:
                            for i in (0, 1):
                                kb = 2 * t + i
                                ib = kb - 4 * j
                                r0 = trim0[ib] if ib >= 0 else 0
                                nc.tensor.matmul(pso[:, r0:512],
                                                 veff[:, kb, :],
                                                 se[:, i, r0:512],
                                                 start=(kb == 0),
                                                 stop=(kb == nkb - 1))

                    # square straight from PSUM so the ssq matmul does not
                    # wait on the u_f copy; copy u out right after so the
                    # psum slot frees without waiting on the norm chain
                    # The softmax 1/den, the exp bias, and the RMS rsqrt
                    # fold into a single scale:
                    #   comb = exp(-0.5*ln(ssq/128)) = rsqrt(mean u^2)
                    # (eps*den^2 is ~6e-4 of ssq/128, below fp16 matmul
                    # noise, so den is never computed; rmsw*(1-lam_init) is
                    # folded into wo on the host.)
                    u_f = ufp.tile([D, 512], f32, tag="uf")
                    nc.vector.tensor_copy(u_f, pso)
                    sq = ufp.tile([D, 512], fp16, tag="sq")
                    nc.vector.tensor_mul(sq, u_f, u_f)
                    psss = pso_p.tile([D, 512], f32, tag="o", name="psss")
                    nc.tensor.matmul(psss, ones_t, sq, start=True, stop=True)

                    def norm(j=j, h=h, psss=psss, u_f=u_f, finj=finj):
                        lnr = ctmp.tile([D, 512], f32, tag="lnr")
                        nc.scalar.activation(lnr, psss, Act.Ln,
                                             scale=1.0 / 128.0)
                        comb = ctmp.tile([D, 512], f32, tag="comb")
                        nc.scalar.activation(comb, lnr, Act.Exp, scale=-0.5)
                        nc.vector.tensor_mul(finj[:, h, :], u_f, comb)

                    pending_norm[0] = norm
                    if j == NJ - 1 and h == NH - 1:
                        # last head: emit inline so the tail o_proj (which
                        # reads every head of finalT) starts sooner
                        emit_norm()

                # flush any o_proj tiles the pair loops didn't absorb
                emit_oproj(len(pending_oproj))

                # queue o_proj for this slice (interleaved into the next
                # slice's attention pair loops)
                for lb in range(4 * j, 4 * j + 4):
                    for osl in range(HID // 512):
                        def oproj(j=j, lb=lb, osl=osl):
                            lrel = slice(128 * (lb - 4 * j),
                                         128 * (lb - 4 * j) + 128)
                            osl_s = slice(512 * osl, 512 * (osl + 1))
                            fin = fin_tiles[j]
                            ps = mm_p.tile([D, 512], f32, tag="mm",
                                           name="psop")
                            for h in range(NH):
                                nc.tensor.matmul(ps, fin[:, h, lrel],
                                                 wo_t[:, h, osl_s],
                                                 start=(h == 0),
                                                 stop=(h == NH - 1))
                            ob = dout.tile([D, 512], fp16, tag="ob")
                            # copy engine: during attn(1) DVE is the hot
                            # engine (RoPE) -> use ACT for o_proj(0); during
                            # attn(2,3) ACT is hot (exp) -> DVE; the tail
                            # flush of o_proj(3) alternates
                            if j == 0:
                                use_act = True
                            elif j == 3:
                                use_act = (lb + osl) % 2 == 1
                            else:
                                use_act = False
                            if use_act:
                                nc.scalar.copy(out=ob, in_=ps)
                            else:
                                nc.vector.tensor_copy(ob, ps)
                            nc.sync.dma_start(
                                out=part[128 * lb:128 * (lb + 1), osl_s],
                                in_=ob)
                        pending_oproj.append(oproj)

            # tail: last head's norm, then the last slice's o_proj
            emit_norm()
            emit_oproj(len(pending_oproj))

    nc.finalize()
    return nc


def _causal_masks():
    # [D, 512] = [tri | ones | zeros | tri] for the packed second-diagonal
    # pair (block ib=2 in cols [0:256] of the shared bank, ib=3 in
    # [256:512]); the in-block causal triangle (partition p = k offset,
    # col c = q offset: 1 iff q >= k) at cols [0:128] is reused for the
    # ib=0/1 single-block masks.
    m = np.zeros((D, 512), np.float32)
    for p in range(D):
        m[p, p:128] = 1.0
        m[p, 128:256] = 1.0
        m[p, 384 + p:512] = 1.0
    return m


def kernel(hidden_states, cos, sin, Wq, Wk, Wv, Wo,
           lambda_q1, lambda_k1, lambda_q2, lambda_k2, rms_weight):
    from concourse.bass_utils import run_bass_kernel_spmd

    fp16 = np.float16

    length = hidden_states.shape[1]
    if length not in _CACHE:
        _CACHE[length] = _build(length)
    nc = _CACHE[length]

    hidden_states = np.asarray(hidden_states, np.float32)
    cos = np.asarray(cos, np.float32)
    sin = np.asarray(sin, np.float32)

    lam_full = np.float32(
        np.exp(np.float32(np.dot(np.asarray(lambda_q1, np.float32),
                                 np.asarray(lambda_k1, np.float32)))
               + np.float32(np.dot(np.asarray(lambda_q2, np.float32),
                                   np.asarray(lambda_k2, np.float32))))
        + np.float32(LAMBDA_INIT))
    lam_arr = np.full((D, 1), -lam_full, np.float32)
    masks = _causal_masks().astype(fp16)

    Wq = np.asarray(Wq, np.float32)
    Wk = np.asarray(Wk, np.float32)
    Wv = np.asarray(Wv, np.float32)
    # fold per-head-dim RMSNorm weight and the (1 - lambda_init) factor into
    # Wo's rows (row index r of the per-core Wo slice has head-dim r % D)
    wo_scale = (np.asarray(rms_weight, np.float32)
                * np.float32(1.0 - LAMBDA_INIT))
    Wo = np.asarray(Wo, np.float32) * np.tile(wo_scale, H)[:, None]

    # sign-fold the rotate_half into the sin table: rows 0..63 negated
    sin_fold = np.concatenate([-sin[:, :, 0:64], sin[:, :, 64:128]], axis=2)

    in_maps = []
    for b in range(B):
        hsT_b = np.ascontiguousarray(hidden_states[b].T.astype(fp16))
        # cs[:, 0, :] = cos^T, cs[:, 1, :] = sign-folded sin^T
        cs_b = np.ascontiguousarray(np.stack(
            [cos[b].T, sin_fold[b].T], axis=1).astype(fp16))
        for t in range(4):
            vlo, vhi = t // 2, t // 2 + 2
            in_maps.append({
                "hsT": hsT_b,
                "cs": cs_b,
                "wqkv": np.ascontiguousarray(np.concatenate(
                    [Wq[:, 512 * t:512 * (t + 1)],
                     Wk[:, 128 * t:128 * (t + 1)],
                     Wv[:, 128 * vlo:128 * (vlo + 1)],
                     Wv[:, 128 * vhi:128 * (vhi + 1)]],
                    axis=1).astype(fp16)),
                "wo": np.ascontiguousarray(
                    Wo[512 * t:512 * (t + 1), :].astype(fp16)),
                "lam": lam_arr,
                "masks": masks,
            })

    trace = bool(os.environ.get("DIFFATTN_TRACE"))
    res = run_bass_kernel_spmd(nc, in_maps, list(range(8)), trace=trace)
    kernel.last_results = res

    out = np.empty((B, length, HID), np.float32)
    for b in range(B):
        acc = res.results[4 * b]["part"].astype(np.float32)
        for t in range(1, 4):
            acc = acc + res.results[4 * b + t]["part"].astype(np.float32)
        out[b] = acc
    return out


# revision 27
# speedup vs baseline: 1.0453x; 1.0453x over previous
"""Differential-Transformer attention (DiffAttn) Trainium2 Bass kernel.

Sharding: 8 cores = 2 (batch) x 4 (head-group tensor parallel).
Core c = 4*b + t handles batch b, query heads 4t..4t+3, kv head t,
and the two v-heads its query heads need (t//2 and t//2+2).
o_proj is row-parallel: each core returns a partial [L, HID] product;
the host sums the 4 partials per batch (the "unshard" step).

The datapath runs in fp16 (11-bit mantissa: same accuracy class as
the PE's fp32r mode, measured mean rel err ~5e-3 for either), which
halves DMA bytes and SBUF footprint, runs matmuls at full PE speed,
and -- unlike fp32r -- is a native dtype, so DMA can feed matmuls
directly with no on-chip rounding passes.  All value magnitudes here
are < 1e3 and weights are ~0.02-scale, far from fp16 range limits;
the unnormalized-softmax exp gets a constant -2 bias for overflow
headroom, which cancels exactly in the RMS-norm fold below.

Structure: ONE fused pipeline over the four 512-wide q-slices:
projections+RoPE (PE/DVE-heavy, no ACT) for slice j run right before
attention for slice j (ACT-heavy exp), and o_proj tiles for slice
j-1 are interleaved into attention j's score/AV pair loop to keep PE
fed where it would otherwise wait on ACT's exp.  Each head's
normalization (Ln/Exp/scale) is deferred into the next head's stream
so the ACT queue never blocks the next head's first exp.

Softmax runs without max-subtraction (score magnitudes are bounded
~5) on transposed score tiles S^T[k, q] so the AV matmul needs no
transposes.  Score/AV matmuls and the exp are trimmed to the causal
triangle at 128-block granularity (256-row floor on moving dims,
below which the PE runs at 1/4 throughput anyway).  RoPE: the sin
table is sign-folded on the host (rows 0..63 negated), so
q' = q*cos + swap(q)*sin_folded needs three muls and one add, with
the partition swap read from PSUM (PSUM+SBUF input pairs are exempt
from the same-base-partition rule).  The softmax denominator and the
RMS-norm rsqrt fold into one scale from an all-ones stationary
matmul (sum of squares replicated across partitions):
  comb = exp(-0.5*ln(ssq/128))  (= rsqrt(mean u^2); the softmax 1/den
and the exp bias cancel exactly inside the RMS norm, and eps*den^2
is negligible).  The RMSNorm weight and the (1-lambda_init) factor
are folded into Wo on the host.
"""

import os
import sys

import numpy as np

for _p in ("/opt/trn_rl_repo",):
    if _p not in sys.path and os.path.isdir(_p):
        sys.path.insert(0, _p)

B = 2
L = 2048
HID = 2048
D = 128
H = 16
NH = 4            # query heads per core
CT = HID // 128   # contraction tiles for the projections
EPS = 1e-6
LAMBDA_INIT = 0.2
EXP_BIAS = -2.0   # overflow headroom for fp16 exp; cancels in the norm

_CACHE = {}


def _build(length=L):
    from concourse import bacc
    import concourse.mybir as mybir
    import concourse.tile as tile

    f32 = mybir.dt.float32
    fp16 = mybir.dt.float16
    Act = mybir.ActivationFunctionType

    # Both ACT functions used here (Exp, Ln) live together in the
    # 'natural_log_exp_and_others' LUT set, but the table-load pass assigns
    # each activation the first set containing its function, which alternates
    # sets and inserts a ~1.3us table reload per switch.  Restrict the pass's
    # view to that one set -> exactly one load total.
    _orig_tables = bacc.get_activation_tables

    def _only_ln_exp(arch):
        t = _orig_tables(arch)
        keep = "natural_log_exp_and_others"
        if keep not in t:
            return t
        return {name: (s if name == keep else set()) for name, s in t.items()}

    bacc.get_activation_tables = _only_ln_exp
    try:
        return _build_inner(length, bacc, mybir, tile, f32, fp16, Act)
    finally:
        bacc.get_activation_tables = _orig_tables


def _build_inner(length, bacc, mybir, tile, f32, fp16, Act):
    NJ = length // 512    # q-slices
    NLB = length // 128   # l/k blocks

    nc = bacc.Bacc()
    hsT = nc.dram_tensor("hsT", [HID, length], fp16, kind="ExternalInput")
    cs = nc.dram_tensor("cs", [D, 2, length], fp16, kind="ExternalInput")
    wqkv = nc.dram_tensor("wqkv", [HID, 7 * D], fp16, kind="ExternalInput")
    wo = nc.dram_tensor("wo", [NH * D, HID], fp16, kind="ExternalInput")
    lam = nc.dram_tensor("lam", [D, 1], f32, kind="ExternalInput")
    masks = nc.dram_tensor("masks", [D, 512], fp16, kind="ExternalInput")
    part = nc.dram_tensor("part", [length, HID], fp16, kind="ExternalOutput")

    inv_sqrt_d = 1.0 / np.sqrt(np.float32(D))
    # per-block start column of the trimmed score/AV region (within the
    # 512-wide q slice), floored at 256 moving rows
    trim0 = (0, 128, 256, 256)

    with tile.TileContext(nc) as tc:
        with tc.tile_pool(name="persist", bufs=1) as persist, \
             tc.tile_pool(name="qTp", bufs=2) as qTp, \
             tc.tile_pool(name="finp", bufs=2) as finp, \
             tc.tile_pool(name="hsp", bufs=6) as hsp, \
             tc.tile_pool(name="csp", bufs=2) as csp, \
             tc.tile_pool(name="btmp", bufs=2) as btmp, \
             tc.tile_pool(name="sep", bufs=3) as sep, \
             tc.tile_pool(name="ufp", bufs=2) as ufp, \
             tc.tile_pool(name="ctmp", bufs=2) as ctmp, \
             tc.tile_pool(name="dout", bufs=4) as dout, \
             tc.tile_pool(name="mm_p", bufs=2, space="PSUM") as mm_p, \
             tc.tile_pool(name="pss_p", bufs=2, space="PSUM") as pss_p, \
             tc.tile_pool(name="pso_p", bufs=2, space="PSUM") as pso_p:

            kT = persist.tile([D, length], fp16, tag="kT")
            veff = persist.tile([D, NLB, D], fp16, tag="veff")
            wqkv_t = persist.tile([D, CT, 7 * D], fp16, tag="wqkv")
            wo_t = persist.tile([D, NH, HID], fp16, tag="wo")
            mask_t = persist.tile([D, 512], fp16, tag="mask")
            ebias_t = persist.tile([D, 1], f32, tag="ebias")
            lam_t = persist.tile([D, 1], f32, tag="lam")
            ones_t = persist.tile([D, D], fp16, tag="ones")

            # ---------------- preamble loads ------------------------------
            nc.sync.dma_start(out=lam_t, in_=lam[:, :])
            nc.sync.dma_start(out=mask_t, in_=masks[:, :])
            nc.vector.memset(ones_t, 1.0)
            nc.vector.memset(ebias_t, EXP_BIAS)

            wqkv_r = wqkv.rearrange("(c p) m -> p c m", p=D)
            wo_r = wo.rearrange("(h p) m -> p h m", p=D)
            hs_r = hsT.rearrange("(c p) m -> p c m", p=D)

            def load_hs_slice(j):
                tiles = []
                for g in range(4):
                    t = hsp.tile([D, 4, 512], fp16, tag="hs",
                                 name=f"hs_{j}_{g}")
                    nc.sync.dma_start(
                        out=t, in_=hs_r[:, 4 * g:4 * (g + 1),
                                        512 * j:512 * (j + 1)])
                    for i in range(4):
                        tiles.append(t[:, i, :])
                return tiles

            def load_cs(j):
                t = csp.tile([D, 2, 512], fp16, tag="cs", name=f"cs_{j}")
                nc.sync.dma_start(out=t,
                                  in_=cs[:, :, 512 * j:512 * (j + 1)])
                return t

            hs_tiles = {}
            cs_tiles = {}
            # startup: weight loads issue from the scalar engine's DGE queue
            # while hs loads issue from sync -- the two queues' transfers
            # interleave on the DMA engines, roughly doubling the effective
            # supply rate during the DMA-bound first slice
            for g in range(8):
                gc = slice(2 * g, 2 * (g + 1))
                nc.scalar.dma_start(out=wqkv_t[:, gc, :], in_=wqkv_r[:, gc, :])
            aps0 = []
            for g in range(8):
                t = hsp.tile([D, 2, 512], fp16, tag="hs0", bufs=8,
                             name=f"hs_0_{g}")
                nc.sync.dma_start(out=t,
                                  in_=hs_r[:, 2 * g:2 * (g + 1), 0:512])
                aps0 += [t[:, 0, :], t[:, 1, :]]
            hs_tiles[0] = aps0
            cs_tiles[0] = load_cs(0)
            cs_tiles[1] = load_cs(1)
            hs_tiles[1] = load_hs_slice(1)
            for h in range(NH):
                nc.scalar.dma_start(out=wo_t[:, h, :], in_=wo_r[:, h, :])

            pending_oproj = []   # emit-closures for o_proj tiles
            pending_norm = [None]
            pair_ctr = [0]

            def emit_norm():
                if pending_norm[0] is not None:
                    pending_norm[0]()
                    pending_norm[0] = None

            def emit_oproj(n=1):
                for _ in range(min(n, len(pending_oproj))):
                    pending_oproj.pop(0)()

            fin_tiles = {}

            for j in range(NJ):
                sl = slice(512 * j, 512 * (j + 1))
                # prefetch next cs and the hs slice after next
                if j + 2 < NJ:
                    cs_tiles[j + 2] = load_cs(j + 2)
                    hs_tiles[j + 2] = load_hs_slice(j + 2)
                chunks = hs_tiles.pop(j)
                cs_s = cs_tiles.pop(j)

                # ---------------- projections + RoPE ----------------------
                qTj = qTp.tile([D, NH, 512], fp16, tag="qT", name=f"qT_{j}")

                def rope(ps, db, qTj=qTj, cs_s=cs_s, sl=sl):
                    t1 = btmp.tile([D, 512], fp16, tag="t1")
                    t2 = btmp.tile([D, 512], fp16, tag="t2")
                    nc.vector.tensor_mul(t1, ps, cs_s[:, 0, :])
                    nc.vector.tensor_mul(t2[0:64, :], ps[64:128, :],
                                         cs_s[0:64, 1, :])
                    nc.vector.tensor_mul(t2[64:128, :], ps[0:64, :],
                                         cs_s[64:128, 1, :])
                    dst = qTj[:, db, :] if db < NH else kT[:, sl]
                    nc.vector.tensor_add(dst, t1, t2)

                def vcomb(psv256, i, j=j):
                    # veff = v_lo + (-lam)*v_hi (lam negated on the host)
                    tv = btmp.tile([D, D], f32, tag="tv")
                    nc.vector.tensor_scalar_mul(tv, psv256[:, 128:256], lam_t)
                    nc.vector.tensor_add(veff[:, 4 * j + i, :],
                                         psv256[:, 0:128], tv)

                if j == 0:
                    # slice 0 is paced by the input DMA: walk chunk-major so
                    # the PE does ALL of each chunk's work (5 projection rows
                    # + 4 v accumulations) the moment it lands, using the
                    # still-idle attention PSUM banks as extra accumulators
                    psA = pss_p.tile([D, 2, 512], f32, tag="s", name="p0a")
                    psB = pss_p.tile([D, 2, 512], f32, tag="s", name="p0b")
                    psC = pso_p.tile([D, 512], f32, tag="o", name="p0c")
                    accs = [psA[:, 0, :], psA[:, 1, :], psB[:, 0, :],
                            psB[:, 1, :], psC]
                    for c in range(CT - 1):
                        for db in range(NH + 1):
                            nc.tensor.matmul(
                                accs[db],
                                wqkv_t[:, c, 128 * db:128 * (db + 1)],
                                chunks[c], start=(c == 0), stop=False,
                                skip_group_check=True)
                    # last chunk per-db, k first, each RoPE drain overlapped
                    # with a v accumulation chain on the PE
                    c = CT - 1
                    vq = []
                    for n, db in enumerate((NH, 0, 1, 2, 3)):
                        nc.tensor.matmul(
                            accs[db], wqkv_t[:, c, 128 * db:128 * (db + 1)],
                            chunks[c], start=False, stop=True,
                            skip_group_check=True)
                        rope(accs[db], db)
                        if n >= 1:
                            i = n - 1
                            psv = mm_p.tile([D, 512], f32, tag="mm")
                            for cc in range(CT):
                                nc.tensor.matmul(
                                    psv[:, 0:256],
                                    chunks[cc][:, 128 * i:128 * (i + 1)],
                                    wqkv_t[:, cc, 640:896],
                                    start=(cc == 0), stop=(cc == CT - 1))
                            vq.append((psv, i))
                    for psv, i in vq:
                        vcomb(psv[:, 0:512], i)
                else:
                    for db in range(NH + 1):
                        ps = mm_p.tile([D, 512], f32, tag="mm")
                        for c in range(CT):
                            nc.tensor.matmul(
                                ps, wqkv_t[:, c, 128 * db:128 * (db + 1)],
                                chunks[c],
                                start=(c == 0), stop=(c == CT - 1))
                        rope(ps, db)
                    for i in range(4):
                        psv = mm_p.tile([D, 512], f32, tag="mm")
                        for c in range(CT):
                            nc.tensor.matmul(
                                psv[:, 0:256],
                                chunks[c][:, 128 * i:128 * (i + 1)],
                                wqkv_t[:, c, 640:896],
                                start=(c == 0), stop=(c == CT - 1))
                        vcomb(psv[:, 0:512], i)

                # ---------------- attention -------------------------------
                finj = finp.tile([D, NH, 512], fp16, tag="fin",
                                 name=f"fin_{j}")
                fin_tiles[j] = finj
                nkb = 4 * j + 4
                npair = nkb // 2
                # pace pending o_proj pops across ALL this slice's pair
                # iterations so the last heads aren't starved of PE filler
                slots = NH * max(npair - 1, 1)
                pop_frac = [0.0]
                pop_rate = len(pending_oproj) / slots if slots else 0.0
                for h in range(NH):
                    pso = pso_p.tile([D, 512], f32, tag="o")
                    se_l = {}

                    def score_pair(t, j=j, h=h):
                        pss = pss_p.tile([D, 2, 512], f32, tag="s")
                        se = sep.tile([D, 2, 512], fp16, tag="se")
                        ib0 = 2 * t - 4 * j
                        if ib0 == 2:
                            # second diagonal pair: both blocks cover q
                            # [256:512] -- pack block ib=2's scores into
                            # columns [0:256] and ib=3's into [256:512] of
                            # ONE psum bank, so a single contiguous exp and
                            # a single [tri|ones|zero|tri] mask cover both
                            nc.tensor.matmul(
                                pss[:, 0, 0:256],
                                kT[:, 128 * (4 * j + 2):128 * (4 * j + 3)],
                                qTj[:, h, 256:512], start=True, stop=True)
                            nc.tensor.matmul(
                                pss[:, 0, 256:512],
                                kT[:, 128 * (4 * j + 3):128 * (4 * j + 4)],
                                qTj[:, h, 256:512], start=True, stop=True)
                            nc.scalar.activation(
                                se[:, 0, :], pss[:, 0, :], Act.Exp,
                                bias=ebias_t, scale=float(inv_sqrt_d))
                            nc.vector.tensor_mul(se[:, 0, :], se[:, 0, :],
                                                 mask_t)
                        else:
                            for i in (0, 1):
                                kb = 2 * t + i
                                ib = kb - 4 * j   # in-slice block index
                                r0 = trim0[ib] if ib >= 0 else 0
                                nc.tensor.matmul(
                                    pss[:, i, r0:512],
                                    kT[:, 128 * kb:128 * (kb + 1)],
                                    qTj[:, h, r0:512],
                                    start=True, stop=True)
                                if ib < 0 and i == 1:
                                    # non-diagonal pair: one exp for both
                                    nc.scalar.activation(
                                        se, pss, Act.Exp,
                                        bias=ebias_t, scale=float(inv_sqrt_d))
                                elif ib >= 0:
                                    nc.scalar.activation(
                                        se[:, i, r0:512], pss[:, i, r0:512],
                                        Act.Exp,
                                        bias=ebias_t, scale=float(inv_sqrt_d))
                                    # mask the diagonal 128-block
                                    m0 = 128 * ib
                                    nc.vector.tensor_mul(
                                        se[:, i, m0:m0 + 128],
                                        se[:, i, m0:m0 + 128],
                                        mask_t[:, 0:128])
                        se_l[t] = se

                    score_pair(0)
                    for t in range(npair):
                        if t + 1 < npair:
                            score_pair(t + 1)
                            if t == 0:
                                # previous head's normalization: emitted two
                                # exps deep so its Ln (waiting on the ssq
                                # matmul) never stalls the ACT queue head
                                emit_norm()
                            pop_frac[0] += pop_rate
                            if pop_frac[0] >= 1.0:
                                pop_frac[0] -= 1.0
                                emit_oproj(1)
                        se = se_l.pop(t)
                        if 2 * t - 4 * j == 2:
                            nc.tensor.matmul(pso[:, 256:512],
                                             veff[:, 4 * j + 2, :],
                                             se[:, 0, 0:256],
                                             start=False, stop=False)
                            nc.tensor.matmul(pso[:, 256:512],
                                             veff[:, 4 * j + 3, :],
                                             se[:, 0, 256:512],
                                             start=False, stop=True)
                        else:
                            for i in (0, 1):
                                kb = 2 * t + i
                                ib = kb - 4 * j
                                r0 = trim0[ib] if ib >= 0 else 0
                                nc.tensor.matmul(pso[:, r0:512],
                                                 veff[:, kb, :],
                                                 se[:, i, r0:512],
                                                 start=(kb == 0),
                                                 stop=(kb == nkb - 1))

                    # square straight from PSUM so the ssq matmul does not
                    # wait on the u_f copy; copy u out right after so the
                    # psum slot frees without waiting on the norm chain
                    # The softmax 1/den, the exp bias, and the RMS rsqrt
                    # fold into a single scale:
                    #   comb = exp(-0.5*ln(ssq/128)) = rsqrt(mean u^2)
                    # (eps*den^2 is ~6e-4 of ssq/128, below fp16 matmul
                    # noise, so den is never computed; rmsw*(1-lam_init) is
                    # folded into wo on the host.)
                    u_f = ufp.tile([D, 512], f32, tag="uf")
                    nc.vector.tensor_copy(u_f, pso)
                    sq = ufp.tile([D, 512], fp16, tag="sq")
                    nc.vector.tensor_mul(sq, u_f, u_f)
                    psss = pso_p.tile([D, 512], f32, tag="o", name="psss")
                    nc.tensor.matmul(psss, ones_t, sq, start=True, stop=True)

                    def norm(j=j, h=h, psss=psss, u_f=u_f, finj=finj):
                        lnr = ctmp.tile([D, 512], f32, tag="lnr")
                        nc.scalar.activation(lnr, psss, Act.Ln,
                                             scale=1.0 / 128.0)
                        comb = ctmp.tile([D, 512], f32, tag="comb")
                        nc.scalar.activation(comb, lnr, Act.Exp, scale=-0.5)
                        nc.vector.tensor_mul(finj[:, h, :], u_f, comb)

                    pending_norm[0] = norm
                    if j == NJ - 1 and h == NH - 1:
                        # last head: emit inline so the tail o_proj (which
                        # reads every head of finalT) starts sooner
                        emit_norm()

                # flush any o_proj tiles the pair loops didn't absorb
                emit_oproj(len(pending_oproj))

                # queue o_proj for this slice (interleaved into the next
                # slice's attention pair loops)
                for lb in range(4 * j, 4 * j + 4):
                    for osl in range(HID // 512):
                        def oproj(j=j, lb=lb, osl=osl):
                            lrel = slice(128 * (lb - 4 * j),
                                         128 * (lb - 4 * j) + 128)
                            osl_s = slice(512 * osl, 512 * (osl + 1))
                            fin = fin_tiles[j]
                            ps = mm_p.tile([D, 512], f32, tag="mm",
                                           name="psop")
                            for h in range(NH):
                                nc.tensor.matmul(ps, fin[:, h, lrel],
                                                 wo_t[:, h, osl_s],
                                                 start=(h == 0),
                                                 stop=(h == NH - 1))
                            ob = dout.tile([D, 512], fp16, tag="ob")
                            # copy engine: during attn(1) DVE is the hot
                            # engine (RoPE) -> use ACT for o_proj(0); during
                            # attn(2,3) ACT is hot (exp) -> DVE; the tail
                            # flush of o_proj(3) alternates
                            if j == 0:
                                use_act = True
                            elif j == 3:
                                use_act = (lb + osl) % 2 == 1
                            else:
                                use_act = False
                            if use_act:
                                nc.scalar.copy(out=ob, in_=ps)
                            else:
                                nc.vector.tensor_copy(ob, ps)
                            nc.sync.dma_start(
                                out=part[128 * lb:128 * (lb + 1), osl_s],
                                in_=ob)
                        pending_oproj.append(oproj)

            # tail: last head's norm, then the last slice's o_proj
            emit_norm()
            emit_oproj(len(pending_oproj))

    nc.finalize()
    return nc


def _causal_masks():
    # [D, 512] = [tri | ones | zeros | tri] for the packed second-diagonal
    # pair (block ib=2 in cols [0:256] of the shared bank, ib=3 in
    # [256:512]); the in-block causal triangle (partition p = k offset,
    # col c = q offset: 1 iff q >= k) at cols [0:128] is reused for the
    # ib=0/1 single-block masks.
    m = np.zeros((D, 512), np.float32)
    for p in range(D):
        m[p, p:128] = 1.0
        m[p, 128:256] = 1.0
        m[p, 384 + p:512] = 1.0
    return m


def kernel(hidden_states, cos, sin, Wq, Wk, Wv, Wo,
           lambda_q1, lambda_k1, lambda_q2, lambda_k2, rms_weight):
    from concourse.bass_utils import run_bass_kernel_spmd

    fp16 = np.float16

    length = hidden_states.shape[1]
    if length not in _CACHE:
        _CACHE[length] = _build(length)
    nc = _CACHE[length]

    hidden_states = np.asarray(hidden_states, np.float32)
    cos = np.asarray(cos, np.float32)
    sin = np.asarray(sin, np.float32)

    lam_full = np.float32(
        np.exp(np.float32(np.dot(np.asarray(lambda_q1, np.float32),
                                 np.asarray(lambda_k1, np.float32)))
               + np.float32(np.dot(np.asarray(lambda_q2, np.float32),
                                   np.asarray(lambda_k2, np.float32))))
        + np.float32(LAMBDA_INIT))
    lam_arr = np.full((D, 1), -lam_full, np.float32)
    masks = _causal_masks().astype(fp16)

    Wq = np.asarray(Wq, np.float32)
    Wk = np.asarray(Wk, np.float32)
    Wv = np.asarray(Wv, np.float32)
    # fold per-head-dim RMSNorm weight and the (1 - lambda_init) factor into
    # Wo's rows (row index r of the per-core Wo slice has head-dim r % D)
    wo_scale = (np.asarray(rms_weight, np.float32)
                * np.float32(1.0 - LAMBDA_INIT))
    Wo = np.asarray(Wo, np.float32) * np.tile(wo_scale, H)[:, None]

    # sign-fold the rotate_half into the sin table: rows 0..63 negated
    sin_fold = np.concatenate([-sin[:, :, 0:64], sin[:, :, 64:128]], axis=2)

    in_maps = []
    for b in range(B):
        hsT_b = np.ascontiguousarray(hidden_states[b].T.astype(fp16))
        # cs[:, 0, :] = cos^T, cs[:, 1, :] = sign-folded sin^T
        cs_b = np.ascontiguousarray(np.stack(
            [cos[b].T, sin_fold[b].T], axis=1).astype(fp16))
        for t in range(4):
            vlo, vhi = t // 2, t // 2 + 2
            in_maps.append({
                "hsT": hsT_b,
                "cs": cs_b,
                "wqkv": np.ascontiguousarray(np.concatenate(
                    [Wq[:, 512 * t:512 * (t + 1)],
                     Wk[:, 128 * t:128 * (t + 1)],
                     Wv[:, 128 * vlo:128 * (vlo + 1)],
                     Wv[:, 128 * vhi:128 * (vhi + 1)]],
                    axis=1).astype(fp16)),
                "wo": np.ascontiguousarray(
                    Wo[512 * t:512 * (t + 1), :].astype(fp16)),
                "lam": lam_arr,
                "masks": masks,
            })

    trace = bool(os.environ.get("DIFFATTN_TRACE"))
    res = run_bass_kernel_spmd(nc, in_maps, list(range(8)), trace=trace)
    kernel.last_results = res

    out = np.empty((B, length, HID), np.float32)
    for b in range(B):
        acc = res.results[4 * b]["part"].astype(np.float32)
        for t in range(1, 4):
            acc = acc + res.results[4 * b + t]["part"].astype(np.float32)
        out[b] = acc
    return out


# revision 39
# speedup vs baseline: 1.0501x; 1.0047x over previous
"""Differential-Transformer attention (DiffAttn) Trainium2 Bass kernel.

Sharding: 8 cores = 2 (batch) x 4 (head-group tensor parallel).
Core c = 4*b + t handles batch b, query heads 4t..4t+3, kv head t,
and the two v-heads its query heads need (t//2 and t//2+2).
o_proj is row-parallel: each core returns a partial [L, HID] product;
the host sums the 4 partials per batch (the "unshard" step).

The datapath runs in fp16 (11-bit mantissa: same accuracy class as
the PE's fp32r mode, measured mean rel err ~5e-3 for either), which
halves DMA bytes and SBUF footprint, runs matmuls at full PE speed,
and -- unlike fp32r -- is a native dtype, so DMA can feed matmuls
directly with no on-chip rounding passes.  All value magnitudes here
are < 1e3 and weights are ~0.02-scale, far from fp16 range limits;
the unnormalized-softmax exp gets a constant -2 bias for overflow
headroom, which cancels exactly in the RMS-norm fold below.

Structure: ONE fused pipeline over the four 512-wide q-slices:
projections+RoPE (PE/DVE-heavy, no ACT) for slice j run right before
attention for slice j (ACT-heavy exp), and o_proj tiles for slice
j-1 are interleaved into attention j's score/AV pair loop to keep PE
fed where it would otherwise wait on ACT's exp.  Each head's
normalization (Ln/Exp/scale) is deferred into the next head's stream
so the ACT queue never blocks the next head's first exp.

Softmax runs without max-subtraction (score magnitudes are bounded
~5) on transposed score tiles S^T[k, q] so the AV matmul needs no
transposes.  Score/AV matmuls and the exp are trimmed to the causal
triangle at 128-block granularity (256-row floor on moving dims,
below which the PE runs at 1/4 throughput anyway).  RoPE: the sin
table is sign-folded on the host (rows 0..63 negated), so
q' = q*cos + swap(q)*sin_folded needs three muls and one add, with
the partition swap read from PSUM (PSUM+SBUF input pairs are exempt
from the same-base-partition rule).  The softmax denominator and the
RMS-norm rsqrt fold into one scale from an all-ones stationary
matmul (sum of squares replicated across partitions):
  comb = exp(-0.5*ln(ssq/128))  (= rsqrt(mean u^2); the softmax 1/den
and the exp bias cancel exactly inside the RMS norm, and eps*den^2
is negligible).  The RMSNorm weight and the (1-lambda_init) factor
are folded into Wo on the host.
"""

import os
import sys

import numpy as np

for _p in ("/opt/trn_rl_repo",):
    if _p not in sys.path and os.path.isdir(_p):
        sys.path.insert(0, _p)

B = 2
L = 2048
HID = 2048
D = 128
H = 16
NH = 4            # query heads per core
CT = HID // 128   # contraction tiles for the projections
EPS = 1e-6
LAMBDA_INIT = 0.2
EXP_BIAS = -2.0   # overflow headroom for fp16 exp; cancels in the norm

_CACHE = {}


def _build(length=L):
    from concourse import bacc
    import concourse.mybir as mybir
    import concourse.tile as tile

    f32 = mybir.dt.float32
    fp16 = mybir.dt.float16
    Act = mybir.ActivationFunctionType

    # Both ACT functions used here (Exp, Ln) live together in the
    # 'natural_log_exp_and_others' LUT set, but the table-load pass assigns
    # each activation the first set containing its function, which alternates
    # sets and inserts a ~1.3us table reload per switch.  Restrict the pass's
    # view to that one set -> exactly one load total.
    _orig_tables = bacc.get_activation_tables

    def _only_ln_exp(arch):
        t = _orig_tables(arch)
        keep = "natural_log_exp_and_others"
        if keep not in t:
            return t
        return {name: (s if name == keep else set()) for name, s in t.items()}

    bacc.get_activation_tables = _only_ln_exp
    try:
        return _build_inner(length, bacc, mybir, tile, f32, fp16, Act)
    finally:
        bacc.get_activation_tables = _orig_tables


def _build_inner(length, bacc, mybir, tile, f32, fp16, Act):
    NJ = length // 512    # q-slices
    NLB = length // 128   # l/k blocks

    nc = bacc.Bacc()
    hsT = nc.dram_tensor("hsT", [HID, length], fp16, kind="ExternalInput")
    cs = nc.dram_tensor("cs", [D, 2, length], fp16, kind="ExternalInput")
    wqkv = nc.dram_tensor("wqkv", [HID, 7 * D], fp16, kind="ExternalInput")
    wo = nc.dram_tensor("wo", [NH * D, HID], fp16, kind="ExternalInput")
    lam = nc.dram_tensor("lam", [D, 1], f32, kind="ExternalInput")
    masks = nc.dram_tensor("masks", [D, 512], fp16, kind="ExternalInput")
    part = nc.dram_tensor("part", [length, HID], fp16, kind="ExternalOutput")

    inv_sqrt_d = 1.0 / np.sqrt(np.float32(D))
    # per-block start column of the trimmed score/AV region (within the
    # 512-wide q slice), floored at 256 moving rows
    trim0 = (0, 128, 256, 256)

    with tile.TileContext(nc) as tc:
        with tc.tile_pool(name="persist", bufs=1) as persist, \
             tc.tile_pool(name="qTp", bufs=2) as qTp, \
             tc.tile_pool(name="finp", bufs=2) as finp, \
             tc.tile_pool(name="hsp", bufs=6) as hsp, \
             tc.tile_pool(name="csp", bufs=2) as csp, \
             tc.tile_pool(name="btmp", bufs=3) as btmp, \
             tc.tile_pool(name="sep", bufs=4) as sep, \
             tc.tile_pool(name="ufp", bufs=2) as ufp, \
             tc.tile_pool(name="ctmp", bufs=2) as ctmp, \
             tc.tile_pool(name="dout", bufs=4) as dout, \
             tc.tile_pool(name="mm_p", bufs=2, space="PSUM") as mm_p, \
             tc.tile_pool(name="pss_p", bufs=2, space="PSUM") as pss_p, \
             tc.tile_pool(name="pso_p", bufs=2, space="PSUM") as pso_p:

            kT = persist.tile([D, length], fp16, tag="kT")
            veff = persist.tile([D, NLB, D], fp16, tag="veff")
            wqkv_t = persist.tile([D, CT, 7 * D], fp16, tag="wqkv")
            wo_t = persist.tile([D, NH, HID], fp16, tag="wo")
            mask_t = persist.tile([D, 512], fp16, tag="mask")
            ebias_t = persist.tile([D, 1], f32, tag="ebias")
            lam_t = persist.tile([D, 1], f32, tag="lam")
            ones_t = persist.tile([D, D], fp16, tag="ones")

            # ---------------- preamble loads ------------------------------
            nc.vector.memset(ones_t, 1.0)
            nc.vector.memset(ebias_t, EXP_BIAS)

            wqkv_r = wqkv.rearrange("(c p) m -> p c m", p=D)
            wo_r = wo.rearrange("(h p) m -> p h m", p=D)
            hs_r = hsT.rearrange("(c p) m -> p c m", p=D)

            def load_hs_slice(j):
                tiles = []
                for g in range(4):
                    t = hsp.tile([D, 4, 512], fp16, tag="hs",
                                 name=f"hs_{j}_{g}")
                    nc.sync.dma_start(
                        out=t, in_=hs_r[:, 4 * g:4 * (g + 1),
                                        512 * j:512 * (j + 1)])
                    for i in range(4):
                        tiles.append(t[:, i, :])
                return tiles

            def load_cs(j):
                t = csp.tile([D, 2, 512], fp16, tag="cs", name=f"cs_{j}")
                nc.sync.dma_start(out=t,
                                  in_=cs[:, :, 512 * j:512 * (j + 1)])
                return t

            hs_tiles = {}
            cs_tiles = {}
            # startup: weight loads issue from the scalar engine's DGE queue
            # while hs loads issue from sync -- the two queues' transfers
            # interleave on the DMA engines, roughly doubling the effective
            # supply rate during the DMA-bound first slice
            ladder = (2, 2, 2, 2, 2, 2, 2, 2)
            c0 = 0
            for w in ladder:
                nc.scalar.dma_start(out=wqkv_t[:, c0:c0 + w, :],
                                    in_=wqkv_r[:, c0:c0 + w, :])
                c0 += w
            aps0 = []
            c0 = 0
            for n, w in enumerate(ladder):
                t = hsp.tile([D, 4, 512], fp16, tag="hs0", bufs=8,
                             name=f"hs_0_{n}")
                nc.sync.dma_start(out=t[:, 0:w, :],
                                  in_=hs_r[:, c0:c0 + w, 0:512])
                aps0 += [t[:, i, :] for i in range(w)]
                c0 += w
            hs_tiles[0] = aps0
            cs_tiles[0] = load_cs(0)
            cs_tiles[1] = load_cs(1)
            hs_tiles[1] = load_hs_slice(1)
            for h in range(NH):
                nc.scalar.dma_start(out=wo_t[:, h, :], in_=wo_r[:, h, :])
            nc.scalar.dma_start(out=lam_t, in_=lam[:, :])
            nc.scalar.dma_start(out=mask_t, in_=masks[:, :])

            pending_oproj = []   # emit-closures for o_proj tiles
            pending_norm = [None]
            pair_ctr = [0]

            def emit_norm():
                if pending_norm[0] is not None:
                    pending_norm[0]()
                    pending_norm[0] = None

            def emit_oproj(n=1):
                for _ in range(min(n, len(pending_oproj))):
                    pending_oproj.pop(0)()

            fin_tiles = {}
            qT_tiles = {}
            proj_done = {}   # j -> set of emitted db/v units

            def rope(ps, db, j):
                qTj = qT_tiles[j]
                cs_s = cs_tiles[j]
                t1 = btmp.tile([D, 512], fp16, tag="t1")
                t2 = btmp.tile([D, 512], fp16, tag="t2")
                nc.vector.tensor_mul(t1, ps, cs_s[:, 0, :])
                nc.vector.tensor_mul(t2[0:64, :], ps[64:128, :],
                                     cs_s[0:64, 1, :])
                nc.vector.tensor_mul(t2[64:128, :], ps[0:64, :],
                                     cs_s[64:128, 1, :])
                dst = (qTj[:, db, :] if db < NH
                       else kT[:, 512 * j:512 * (j + 1)])
                nc.vector.tensor_add(dst, t1, t2)

            def vcomb(psv256, i, j):
                # veff = v_lo + (-lam)*v_hi (lam negated on the host), on
                # the otherwise-idle GpSimd engine so the busy DVE queue
                # (RoPE chains) never delays veff for the AV matmuls
                tv = btmp.tile([D, D], f32, tag="tv")
                nc.gpsimd.tensor_scalar(tv, psv256[:, 128:256], lam_t,
                                        None, mybir.AluOpType.mult)
                nc.gpsimd.tensor_add(veff[:, 4 * j + i, :],
                                     psv256[:, 0:128], tv)

            def emit_proj(j, dbs, vs):
                chunks = hs_tiles[j]
                done = proj_done.setdefault(j, set())
                for db in dbs:
                    if db in done:
                        continue
                    done.add(db)
                    ps = mm_p.tile([D, 512], f32, tag="mm")
                    for c in range(CT):
                        nc.tensor.matmul(
                            ps, wqkv_t[:, c, 128 * db:128 * (db + 1)],
                            chunks[c],
                            start=(c == 0), stop=(c == CT - 1))
                    rope(ps, db, j)
                for i in vs:
                    if ("v", i) in done:
                        continue
                    done.add(("v", i))
                    psv = mm_p.tile([D, 512], f32, tag="mm")
                    for c in range(CT):
                        nc.tensor.matmul(
                            psv[:, 0:256],
                            chunks[c][:, 128 * i:128 * (i + 1)],
                            wqkv_t[:, c, 640:896],
                            start=(c == 0), stop=(c == CT - 1))
                    vcomb(psv[:, 0:512], i, j)

            for j in range(NJ):
                sl = slice(512 * j, 512 * (j + 1))
                # prefetch next cs and the hs slice after next
                if j + 2 < NJ:
                    cs_tiles[j + 2] = load_cs(j + 2)
                    hs_tiles[j + 2] = load_hs_slice(j + 2)
                chunks = hs_tiles[j]
                if j not in qT_tiles:
                    qT_tiles[j] = qTp.tile([D, NH, 512], fp16, tag="qT",
                                           name=f"qT_{j}")
                qTj = qT_tiles[j]

                if j == 0:
                    # slice 0 is paced by the input DMA: walk chunk-major so
                    # the PE does ALL of each chunk's work (5 projection rows
                    # + 4 v accumulations) the moment it lands, using the
                    # still-idle attention PSUM banks as extra accumulators
                    psA = pss_p.tile([D, 2, 512], f32, tag="s", name="p0a")
                    psB = pss_p.tile([D, 2, 512], f32, tag="s", name="p0b")
                    psC = pso_p.tile([D, 512], f32, tag="o", name="p0c")
                    accs = [psA[:, 0, :], psA[:, 1, :], psB[:, 0, :],
                            psB[:, 1, :], psC]
                    for c in range(CT - 1):
                        for db in range(NH + 1):
                            nc.tensor.matmul(
                                accs[db],
                                wqkv_t[:, c, 128 * db:128 * (db + 1)],
                                chunks[c], start=(c == 0), stop=False,
                                skip_group_check=True)
                    # last chunk per-db, k first, each RoPE drain overlapped
                    # with a v accumulation chain on the PE
                    c = CT - 1
                    for n, db in enumerate((NH, 0, 1, 2, 3)):
                        nc.tensor.matmul(
                            accs[db], wqkv_t[:, c, 128 * db:128 * (db + 1)],
                            chunks[c], start=False, stop=True,
                            skip_group_check=True)
                        rope(accs[db], db, 0)
                        if n >= 1:
                            i = n - 1
                            psv = mm_p.tile([D, 512], f32, tag="mm")
                            for cc in range(CT):
                                nc.tensor.matmul(
                                    psv[:, 0:256],
                                    chunks[cc][:, 128 * i:128 * (i + 1)],
                                    wqkv_t[:, cc, 640:896],
                                    start=(cc == 0), stop=(cc == CT - 1))
                            vcomb(psv[:, 0:512], i, 0)
                    # while slice 0's RoPE chains drain on DVE, keep the PE
                    # busy with the first two db-chains of slice 1
                    qT_tiles[1] = qTp.tile([D, NH, 512], fp16, tag="qT",
                                           name="qT_1")
                    emit_proj(1, [0, 1], [])
                else:
                    emit_proj(j, range(NH + 1), range(4))

                # ---------------- attention -------------------------------
                finj = finp.tile([D, NH, 512], fp16, tag="fin",
                                 name=f"fin_{j}")
                fin_tiles[j] = finj
                nkb = 4 * j + 4
                npair = nkb // 2
                # pace pending o_proj pops across ALL this slice's pair
                # iterations so the last heads aren't starved of PE filler
                slots = NH * max(npair - 1, 1)
                pop_frac = [0.0]
                pop_rate = len(pending_oproj) / slots if slots else 0.0
                for h in range(NH):
                    pso = pso_p.tile([D, 512], f32, tag="o")
                    se_l = {}

                    def score_pair(t, j=j, h=h):
                        pss = pss_p.tile([D, 2, 512], f32, tag="s")
                        se = sep.tile([D, 2, 512], fp16, tag="se")
                        ib0 = 2 * t - 4 * j
                        if ib0 == 2:
                            # second diagonal pair: both blocks cover q
                            # [256:512] -- pack block ib=2's scores into
                            # columns [0:256] and ib=3's into [256:512] of
                            # ONE psum bank, so a single contiguous exp and
                            # a single [tri|ones|zero|tri] mask cover both
                            nc.tensor.matmul(
                                pss[:, 0, 0:256],
                                kT[:, 128 * (4 * j + 2):128 * (4 * j + 3)],
                                qTj[:, h, 256:512], start=True, stop=True)
                            nc.tensor.matmul(
                                pss[:, 0, 256:512],
                                kT[:, 128 * (4 * j + 3):128 * (4 * j + 4)],
                                qTj[:, h, 256:512], start=True, stop=True)
                            nc.scalar.activation(
                                se[:, 0, :], pss[:, 0, :], Act.Exp,
                                bias=ebias_t, scale=float(inv_sqrt_d))
                            nc.vector.tensor_mul(se[:, 0, :], se[:, 0, :],
                                                 mask_t)
                        else:
                            for i in (0, 1):
                                kb = 2 * t + i
                                ib = kb - 4 * j   # in-slice block index
                                r0 = trim0[ib] if ib >= 0 else 0
                                nc.tensor.matmul(
                                    pss[:, i, r0:512],
                                    kT[:, 128 * kb:128 * (kb + 1)],
                                    qTj[:, h, r0:512],
                                    start=True, stop=True)
                                if ib < 0 and i == 1:
                                    # non-diagonal pair: one exp for both
                                    nc.scalar.activation(
                                        se, pss, Act.Exp,
                                        bias=ebias_t, scale=float(inv_sqrt_d))
                                elif ib >= 0:
                                    nc.scalar.activation(
                                        se[:, i, r0:512], pss[:, i, r0:512],
                                        Act.Exp,
                                        bias=ebias_t, scale=float(inv_sqrt_d))
                                    # mask the diagonal 128-block
                                    m0 = 128 * ib
                                    nc.vector.tensor_mul(
                                        se[:, i, m0:m0 + 128],
                                        se[:, i, m0:m0 + 128],
                                        mask_t[:, 0:128])
                        se_l[t] = se

                    score_pair(0)
                    for t in range(npair):
                        if t + 1 < npair:
                            score_pair(t + 1)
                            if t == 0:
                                # previous head's normalization: emitted two
                                # exps deep so its Ln (waiting on the ssq
                                # matmul) never stalls the ACT queue head
                                emit_norm()
                            pop_frac[0] += pop_rate
                            if pop_frac[0] >= 1.0:
                                pop_frac[0] -= 1.0
                                emit_oproj(1)
                        se = se_l.pop(t)
                        if 2 * t - 4 * j == 2:
                            nc.tensor.matmul(pso[:, 256:512],
                                             veff[:, 4 * j + 2, :],
                                             se[:, 0, 0:256],
                                             start=False, stop=False)
                            nc.tensor.matmul(pso[:, 256:512],
                                             veff[:, 4 * j + 3, :],
                                             se[:, 0, 256:512],
                                             start=False, stop=True)
                        else:
                            for i in (0, 1):
                                kb = 2 * t + i
                                ib = kb - 4 * j
                                r0 = trim0[ib] if ib >= 0 else 0
                                nc.tensor.matmul(pso[:, r0:512],
                                                 veff[:, kb, :],
                                                 se[:, i, r0:512],
                                                 start=(kb == 0),
                                                 stop=(kb == nkb - 1))

                    # square straight from PSUM so the ssq matmul does not
                    # wait on the u_f copy; copy u out right after so the
                    # psum slot frees without waiting on the norm chain
                    # The softmax 1/den, the exp bias, and the RMS rsqrt
                    # fold into a single scale:
                    #   comb = exp(-0.5*ln(ssq/128)) = rsqrt(mean u^2)
                    # (eps*den^2 is ~6e-4 of ssq/128, below fp16 matmul
                    # noise, so den is never computed; rmsw*(1-lam_init) is
                    # folded into wo on the host.)
                    u_f = ufp.tile([D, 512], f32, tag="uf")
                    if j <= 1:
                        # ACT is cool in the early slices; DVE is the hot
                        # engine there (RoPE + norm chains)
                        nc.scalar.copy(out=u_f, in_=pso)
                    else:
                        nc.gpsimd.tensor_copy(out=u_f, in_=pso)
                    sq = ufp.tile([D, 512], fp16, tag="sq")
                    nc.vector.tensor_mul(sq, u_f, u_f)
                    psss = pso_p.tile([D, 512], f32, tag="o", name="psss")
                    nc.tensor.matmul(psss, ones_t, sq, start=True, stop=True)

                    def norm(j=j, h=h, psss=psss, u_f=u_f, finj=finj):
                        lnr = ctmp.tile([D, 512], f32, tag="lnr")
                        nc.scalar.activation(lnr, psss, Act.Ln,
                                             scale=1.0 / 128.0)
                        comb = ctmp.tile([D, 512], f32, tag="comb")
                        nc.scalar.activation(comb, lnr, Act.Exp, scale=-0.5)
                        nc.vector.tensor_mul(finj[:, h, :], u_f, comb)

                    pending_norm[0] = norm
                    if j == NJ - 1 and h == NH - 1:
                        # last head: emit inline so the tail o_proj (which
                        # reads every head of finalT) starts sooner
                        emit_norm()

                # flush any o_proj tiles the pair loops didn't absorb
                emit_oproj(len(pending_oproj))

                # queue o_proj for this slice (interleaved into the next
                # slice's attention pair loops); the last slice is handled
                # by the wave-pipelined tail below
                for lb in range(4 * j, 4 * j + 4) if j < NJ - 1 else []:
                    for osl in range(HID // 512):
                        def oproj(j=j, lb=lb, osl=osl):
                            lrel = slice(128 * (lb - 4 * j),
                                         128 * (lb - 4 * j) + 128)
                            osl_s = slice(512 * osl, 512 * (osl + 1))
                            fin = fin_tiles[j]
                            ps = mm_p.tile([D, 512], f32, tag="mm",
                                           name="psop")
                            for h in range(NH):
                                nc.tensor.matmul(ps, fin[:, h, lrel],
                                                 wo_t[:, h, osl_s],
                                                 start=(h == 0),
                                                 stop=(h == NH - 1))
                            ob = dout.tile([D, 512], fp16, tag="ob")
                            # copy engine: during attn(1) DVE is the hot
                            # engine (RoPE) -> use ACT for o_proj(0); during
                            # attn(2,3) ACT is hot (exp) -> DVE; the tail
                            # flush of o_proj(3) alternates
                            if j == 0:
                                use_act = True
                            elif j == 3:
                                use_act = (lb + osl) % 2 == 1
                            else:
                                use_act = False
                            if use_act:
                                nc.scalar.copy(out=ob, in_=ps)
                            else:
                                nc.vector.tensor_copy(ob, ps)
                            nc.sync.dma_start(
                                out=part[128 * lb:128 * (lb + 1), osl_s],
                                in_=ob)
                        pending_oproj.append(oproj)

            # tail: last head's norm, then the last slice's o_proj in two
            # 8-tile waves across ALL psum banks: each wave accumulates
            # heads 0..2 first (ready before the last head's deferred norm
            # lands), then finishes with head 3 and drains
            emit_norm()
            finj3 = fin_tiles[NJ - 1]
            for wave in range(2):
                regions = [
                    mm_p.tile([D, 512], f32, tag="mm", name=f"tw{wave}a"),
                    mm_p.tile([D, 512], f32, tag="mm", name=f"tw{wave}b")]
                pa = pss_p.tile([D, 2, 512], f32, tag="s", name=f"tw{wave}c")
                pb = pss_p.tile([D, 2, 512], f32, tag="s", name=f"tw{wave}d")
                regions += [pa[:, 0, :], pa[:, 1, :], pb[:, 0, :],
                            pb[:, 1, :],
                            pso_p.tile([D, 512], f32, tag="o",
                                       name=f"tw{wave}e"),
                            pso_p.tile([D, 512], f32, tag="o",
                                       name=f"tw{wave}f")]
                idxs = list(range(8 * wave, 8 * wave + 8))
                for m, idx in enumerate(idxs):
                    lrel = slice(128 * (idx // 4), 128 * (idx // 4) + 128)
                    osl_s = slice(512 * (idx % 4), 512 * (idx % 4) + 512)
                    for h in range(NH - 1):
                        nc.tensor.matmul(regions[m], finj3[:, h, lrel],
                                         wo_t[:, h, osl_s],
                                         start=(h == 0), stop=False,
                                         skip_group_check=True)
                for m, idx in enumerate(idxs):
                    lb = 4 * (NJ - 1) + idx // 4
                    lrel = slice(128 * (idx // 4), 128 * (idx // 4) + 128)
                    osl_s = slice(512 * (idx % 4), 512 * (idx % 4) + 512)
                    nc.tensor.matmul(regions[m], finj3[:, NH - 1, lrel],
                                     wo_t[:, NH - 1, osl_s],
                                     start=False, stop=True,
                                     skip_group_check=True)
                    ob = dout.tile([D, 512], fp16, tag="ob")
                    if m % 2 == 0:
                        nc.vector.tensor_copy(ob, regions[m])
                    else:
                        nc.scalar.copy(out=ob, in_=regions[m])
                    (nc.sync if m % 2 == 0 else nc.scalar).dma_start(
                        out=part[128 * lb:128 * (lb + 1), osl_s], in_=ob)

    nc.finalize()
    return nc


def _causal_masks():
    # [D, 512] = [tri | ones | zeros | tri] for the packed second-diagonal
    # pair (block ib=2 in cols [0:256] of the shared bank, ib=3 in
    # [256:512]); the in-block causal triangle (partition p = k offset,
    # col c = q offset: 1 iff q >= k) at cols [0:128] is reused for the
    # ib=0/1 single-block masks.
    m = np.zeros((D, 512), np.float32)
    for p in range(D):
        m[p, p:128] = 1.0
        m[p, 128:256] = 1.0
        m[p, 384 + p:512] = 1.0
    return m


def kernel(hidden_states, cos, sin, Wq, Wk, Wv, Wo,
           lambda_q1, lambda_k1, lambda_q2, lambda_k2, rms_weight):
    from concourse.bass_utils import run_bass_kernel_spmd

    fp16 = np.float16

    length = hidden_states.shape[1]
    if length not in _CACHE:
        _CACHE[length] = _build(length)
    nc = _CACHE[length]

    hidden_states = np.asarray(hidden_states, np.float32)
    cos = np.asarray(cos, np.float32)
    sin = np.asarray(sin, np.float32)

    lam_full = np.float32(
        np.exp(np.float32(np.dot(np.asarray(lambda_q1, np.float32),
                                 np.asarray(lambda_k1, np.float32)))
               + np.float32(np.dot(np.asarray(lambda_q2, np.float32),
                                   np.asarray(lambda_k2, np.float32))))
        + np.float32(LAMBDA_INIT))
    lam_arr = np.full((D, 1), -lam_full, np.float32)
    masks = _causal_masks().astype(fp16)

    Wq = np.asarray(Wq, np.float32)
    Wk = np.asarray(Wk, np.float32)
    Wv = np.asarray(Wv, np.float32)
    # fold per-head-dim RMSNorm weight and the (1 - lambda_init) factor into
    # Wo's rows (row index r of the per-core Wo slice has head-dim r % D)
    wo_scale = (np.asarray(rms_weight, np.float32)
                * np.float32(1.0 - LAMBDA_INIT))
    Wo = np.asarray(Wo, np.float32) * np.tile(wo_scale, H)[:, None]

    # sign-fold the rotate_half into the sin table: rows 0..63 negated
    sin_fold = np.concatenate([-sin[:, :, 0:64], sin[:, :, 64:128]], axis=2)

    in_maps = []
    for b in range(B):
        hsT_b = np.ascontiguousarray(hidden_states[b].T.astype(fp16))
        # cs[:, 0, :] = cos^T, cs[:, 1, :] = sign-folded sin^T
        cs_b = np.ascontiguousarray(np.stack(
            [cos[b].T, sin_fold[b].T], axis=1).astype(fp16))
        for t in range(4):
            vlo, vhi = t // 2, t // 2 + 2
            in_maps.append({
                "hsT": hsT_b,
                "cs": cs_b,
                "wqkv": np.ascontiguousarray(np.concatenate(
                    [Wq[:, 512 * t:512 * (t + 1)],
                     Wk[:, 128 * t:128 * (t + 1)],
                     Wv[:, 128 * vlo:128 * (vlo + 1)],
                     Wv[:, 128 * vhi:128 * (vhi + 1)]],
                    axis=1).astype(fp16)),
                "wo": np.ascontiguousarray(
                    Wo[512 * t:512 * (t + 1), :].astype(fp16)),
                "lam": lam_arr,
                "masks": masks,
            })

    trace = bool(os.environ.get("DIFFATTN_TRACE"))
    res = run_bass_kernel_spmd(nc, in_maps, list(range(8)), trace=trace)
    kernel.last_results = res

    out = np.empty((B, length, HID), np.float32)
    for b in range(B):
        acc = res.results[4 * b]["part"].astype(np.float32)
        for t in range(1, 4):
            acc = acc + res.results[4 * b + t]["part"].astype(np.float32)
        out[b] = acc
    return out


# revision 51
# speedup vs baseline: 1.0727x; 1.0215x over previous
"""Differential-Transformer attention (DiffAttn) Trainium2 Bass kernel.

Sharding: 8 cores = 2 (batch) x 4 (head-group tensor parallel).
Core c = 4*b + t handles batch b, query heads 4t..4t+3, kv head t,
and the two v-heads its query heads need (t//2 and t//2+2).
o_proj is row-parallel: each core returns a partial [L, HID] product;
the host sums the 4 partials per batch (the "unshard" step).

The datapath runs in fp16 (11-bit mantissa: same accuracy class as
the PE's fp32r mode, measured mean rel err ~5e-3 for either), which
halves DMA bytes and SBUF footprint, runs matmuls at full PE speed,
and -- unlike fp32r -- is a native dtype, so DMA can feed matmuls
directly with no on-chip rounding passes.  All value magnitudes here
are < 1e3 and weights are ~0.02-scale, far from fp16 range limits;
the unnormalized-softmax exp gets a constant -2 bias for overflow
headroom, which cancels exactly in the RMS-norm fold below.

Structure: ONE fused pipeline over the four 512-wide q-slices:
projections+RoPE (PE/DVE-heavy, no ACT) for slice j run right before
attention for slice j (ACT-heavy exp), and o_proj tiles for slice
j-1 are interleaved into attention j's score/AV pair loop to keep PE
fed where it would otherwise wait on ACT's exp.  Each head's
normalization (Ln/Exp/scale) is deferred into the next head's stream
so the ACT queue never blocks the next head's first exp.

Softmax runs without max-subtraction (score magnitudes are bounded
~5) on transposed score tiles S^T[k, q] so the AV matmul needs no
transposes.  Score/AV matmuls and the exp are trimmed to the causal
triangle at 128-block granularity (256-row floor on moving dims,
below which the PE runs at 1/4 throughput anyway).  RoPE: the sin
table is sign-folded on the host (rows 0..63 negated), so
q' = q*cos + swap(q)*sin_folded needs three muls and one add, with
the partition swap read from PSUM (PSUM+SBUF input pairs are exempt
from the same-base-partition rule).  The softmax denominator and the
RMS-norm rsqrt fold into one scale from an all-ones stationary
matmul (sum of squares replicated across partitions):
  comb = exp(-0.5*ln(ssq/128))  (= rsqrt(mean u^2); the softmax 1/den
and the exp bias cancel exactly inside the RMS norm, and eps*den^2
is negligible).  The RMSNorm weight and the (1-lambda_init) factor
are folded into Wo on the host.
"""

import os
import sys

import numpy as np

for _p in ("/opt/trn_rl_repo",):
    if _p not in sys.path and os.path.isdir(_p):
        sys.path.insert(0, _p)

B = 2
L = 2048
HID = 2048
D = 128
H = 16
NH = 4            # query heads per core
CT = HID // 128   # contraction tiles for the projections
EPS = 1e-6
LAMBDA_INIT = 0.2
EXP_BIAS = -2.0   # overflow headroom for fp16 exp; cancels in the norm

_CACHE = {}


def _build(length=L):
    from concourse import bacc
    import concourse.mybir as mybir
    import concourse.tile as tile

    f32 = mybir.dt.float32
    fp16 = mybir.dt.float16
    Act = mybir.ActivationFunctionType

    # Both ACT functions used here (Exp, Ln) live together in the
    # 'natural_log_exp_and_others' LUT set, but the table-load pass assigns
    # each activation the first set containing its function, which alternates
    # sets and inserts a ~1.3us table reload per switch.  Restrict the pass's
    # view to that one set -> exactly one load total.
    _orig_tables = bacc.get_activation_tables

    def _only_ln_exp(arch):
        t = _orig_tables(arch)
        keep = "natural_log_exp_and_others"
        if keep not in t:
            return t
        return {name: (s if name == keep else set()) for name, s in t.items()}

    bacc.get_activation_tables = _only_ln_exp
    try:
        return _build_inner(length, bacc, mybir, tile, f32, fp16, Act)
    finally:
        bacc.get_activation_tables = _orig_tables


def _build_inner(length, bacc, mybir, tile, f32, fp16, Act):
    NJ = length // 512    # q-slices
    NLB = length // 128   # l/k blocks

    nc = bacc.Bacc()
    hsT = nc.dram_tensor("hsT", [HID, length], fp16, kind="ExternalInput")
    cs = nc.dram_tensor("cs", [D, 2, length], fp16, kind="ExternalInput")
    wqkv = nc.dram_tensor("wqkv", [HID, 7 * D], fp16, kind="ExternalInput")
    wo = nc.dram_tensor("wo", [NH * D, HID], fp16, kind="ExternalInput")
    lam = nc.dram_tensor("lam", [D, 1], f32, kind="ExternalInput")
    masks = nc.dram_tensor("masks", [D, 512], fp16, kind="ExternalInput")
    part = nc.dram_tensor("part", [length, HID], fp16, kind="ExternalOutput")

    inv_sqrt_d = 1.0 / np.sqrt(np.float32(D))
    # per-block start column of the trimmed score/AV region (within the
    # 512-wide q slice), floored at 256 moving rows
    trim0 = (0, 128, 256, 256)

    with tile.TileContext(nc) as tc:
        with tc.tile_pool(name="persist", bufs=1) as persist, \
             tc.tile_pool(name="qTp", bufs=2) as qTp, \
             tc.tile_pool(name="finp", bufs=2) as finp, \
             tc.tile_pool(name="hsp", bufs=6) as hsp, \
             tc.tile_pool(name="csp", bufs=2) as csp, \
             tc.tile_pool(name="btmp", bufs=3) as btmp, \
             tc.tile_pool(name="sep", bufs=4) as sep, \
             tc.tile_pool(name="ufp", bufs=2) as ufp, \
             tc.tile_pool(name="ctmp", bufs=2) as ctmp, \
             tc.tile_pool(name="dout", bufs=4) as dout, \
             tc.tile_pool(name="mm_p", bufs=2, space="PSUM") as mm_p, \
             tc.tile_pool(name="pss_p", bufs=1, space="PSUM") as pss_p, \
             tc.tile_pool(name="pso_p", bufs=2, space="PSUM") as pso_p:

            kT = persist.tile([D, length], fp16, tag="kT")
            veff = persist.tile([D, NLB, D], fp16, tag="veff")
            wqkv_t = persist.tile([D, CT, 7 * D], fp16, tag="wqkv")
            wo_t = persist.tile([D, NH, HID], fp16, tag="wo")
            mask_t = persist.tile([D, 512], fp16, tag="mask")
            ebias_t = persist.tile([D, 1], f32, tag="ebias")
            lam_t = persist.tile([D, 1], f32, tag="lam")
            ones_t = persist.tile([D, D], fp16, tag="ones")

            # ---------------- preamble loads ------------------------------
            nc.vector.memset(ones_t, 1.0)
            nc.vector.memset(ebias_t, EXP_BIAS)

            wqkv_r = wqkv.rearrange("(c p) m -> p c m", p=D)
            wo_r = wo.rearrange("(h p) m -> p h m", p=D)
            hs_r = hsT.rearrange("(c p) m -> p c m", p=D)

            def load_hs_slice(j):
                tiles = []
                for g in range(4):
                    t = hsp.tile([D, 4, 512], fp16, tag="hs",
                                 name=f"hs_{j}_{g}")
                    nc.sync.dma_start(
                        out=t, in_=hs_r[:, 4 * g:4 * (g + 1),
                                        512 * j:512 * (j + 1)])
                    for i in range(4):
                        tiles.append(t[:, i, :])
                return tiles

            def load_cs(j):
                t = csp.tile([D, 2, 512], fp16, tag="cs", name=f"cs_{j}")
                nc.sync.dma_start(out=t,
                                  in_=cs[:, :, 512 * j:512 * (j + 1)])
                return t

            hs_tiles = {}
            cs_tiles = {}
            # startup: weight loads issue from the scalar engine's DGE queue
            # while hs loads issue from sync -- the two queues' transfers
            # interleave on the DMA engines, roughly doubling the effective
            # supply rate during the DMA-bound first slice
            ladder = (2, 2, 2, 2, 2, 2, 2, 2)
            c0 = 0
            for w in ladder:
                nc.scalar.dma_start(out=wqkv_t[:, c0:c0 + w, :],
                                    in_=wqkv_r[:, c0:c0 + w, :])
                c0 += w
            aps0 = []
            c0 = 0
            for n, w in enumerate(ladder):
                t = hsp.tile([D, 4, 512], fp16, tag="hs0", bufs=8,
                             name=f"hs_0_{n}")
                nc.sync.dma_start(out=t[:, 0:w, :],
                                  in_=hs_r[:, c0:c0 + w, 0:512])
                aps0 += [t[:, i, :] for i in range(w)]
                c0 += w
            hs_tiles[0] = aps0
            cs_tiles[0] = load_cs(0)
            cs_tiles[1] = load_cs(1)
            hs_tiles[1] = load_hs_slice(1)
            nc.scalar.dma_start(out=lam_t, in_=lam[:, :])
            nc.scalar.dma_start(out=mask_t, in_=masks[:, :])
            for h in range(NH):
                nc.scalar.dma_start(out=wo_t[:, h, :], in_=wo_r[:, h, :])

            pending_oproj = []   # emit-closures for o_proj tiles
            pending_norm = [None]
            pair_ctr = [0]

            def emit_norm():
                if pending_norm[0] is not None:
                    pending_norm[0]()
                    pending_norm[0] = None

            def emit_oproj(n=1):
                for _ in range(min(n, len(pending_oproj))):
                    pending_oproj.pop(0)()

            fin_tiles = {}
            qT_tiles = {}
            proj_done = {}   # j -> set of emitted db/v units

            def rope(ps, db, j):
                qTj = qT_tiles[j]
                cs_s = cs_tiles[j]
                t1 = btmp.tile([D, 512], fp16, tag="t1")
                t2 = btmp.tile([D, 512], fp16, tag="t2")
                nc.vector.tensor_mul(t1, ps, cs_s[:, 0, :])
                nc.vector.tensor_mul(t2[0:64, :], ps[64:128, :],
                                     cs_s[0:64, 1, :])
                nc.vector.tensor_mul(t2[64:128, :], ps[0:64, :],
                                     cs_s[64:128, 1, :])
                dst = (qTj[:, db, :] if db < NH
                       else kT[:, 512 * j:512 * (j + 1)])
                nc.vector.tensor_add(dst, t1, t2)

            def vcomb(psv256, i, j):
                # veff = v_lo + (-lam)*v_hi (lam negated on the host)
                tv = btmp.tile([D, D], f32, tag="tv")
                nc.vector.tensor_scalar_mul(tv, psv256[:, 128:256], lam_t)
                nc.vector.tensor_add(veff[:, 4 * j + i, :],
                                     psv256[:, 0:128], tv)

            def emit_proj(j, dbs, vs):
                chunks = hs_tiles[j]
                done = proj_done.setdefault(j, set())
                for db in dbs:
                    if db in done:
                        continue
                    done.add(db)
                    ps = mm_p.tile([D, 512], f32, tag="mm")
                    for c in range(CT):
                        nc.tensor.matmul(
                            ps, wqkv_t[:, c, 128 * db:128 * (db + 1)],
                            chunks[c],
                            start=(c == 0), stop=(c == CT - 1))
                    rope(ps, db, j)
                for i in vs:
                    if ("v", i) in done:
                        continue
                    done.add(("v", i))
                    psv = mm_p.tile([D, 512], f32, tag="mm")
                    for c in range(CT):
                        nc.tensor.matmul(
                            psv[:, 0:256],
                            chunks[c][:, 128 * i:128 * (i + 1)],
                            wqkv_t[:, c, 640:896],
                            start=(c == 0), stop=(c == CT - 1))
                    vcomb(psv[:, 0:512], i, j)

            for j in range(NJ):
                sl = slice(512 * j, 512 * (j + 1))
                # prefetch next cs and the hs slice after next
                if j + 2 < NJ:
                    cs_tiles[j + 2] = load_cs(j + 2)
                    hs_tiles[j + 2] = load_hs_slice(j + 2)
                chunks = hs_tiles[j]
                if j not in qT_tiles:
                    qT_tiles[j] = qTp.tile([D, NH, 512], fp16, tag="qT",
                                           name=f"qT_{j}")
                qTj = qT_tiles[j]

                if j == 0:
                    # slice 0 is paced by the input DMA: walk chunk-major so
                    # the PE does ALL of each chunk's work (5 projection rows
                    # + 4 v accumulations) the moment it lands, using the
                    # still-idle attention PSUM banks as extra accumulators
                    psA = pss_p.tile([D, 4, 512], f32, tag="s", name="p0a")
                    psC = pso_p.tile([D, 512], f32, tag="o", name="p0c")
                    accs = [psA[:, 0, :], psA[:, 1, :], psA[:, 2, :],
                            psA[:, 3, :], psC]
                    for c in range(CT - 1):
                        for db in range(NH + 1):
                            nc.tensor.matmul(
                                accs[db],
                                wqkv_t[:, c, 128 * db:128 * (db + 1)],
                                chunks[c], start=(c == 0), stop=False,
                                skip_group_check=True)
                    # last chunk per-db, k first, each RoPE drain overlapped
                    # with a v accumulation chain on the PE
                    c = CT - 1
                    for n, db in enumerate((0, NH, 1, 2, 3)):
                        nc.tensor.matmul(
                            accs[db], wqkv_t[:, c, 128 * db:128 * (db + 1)],
                            chunks[c], start=False, stop=True,
                            skip_group_check=True)
                        rope(accs[db], db, 0)
                        if n == 1:
                            # all four v chains right after rope(q0) so the
                            # vcombs sit EARLY in the DVE queue -- attention
                            # slice 0's AV matmuls need every veff block and
                            # must not wait behind the q1..q3 RoPE chains
                            for i in range(4):
                                psv = mm_p.tile([D, 512], f32, tag="mm")
                                for cc in range(CT):
                                    nc.tensor.matmul(
                                        psv[:, 0:256],
                                        chunks[cc][:, 128 * i:128 * (i + 1)],
                                        wqkv_t[:, cc, 640:896],
                                        start=(cc == 0), stop=(cc == CT - 1))
                                vcomb(psv[:, 0:512], i, 0)
                    # while slice 0's RoPE chains drain on DVE, keep the PE
                    # busy with the first two db-chains of slice 1
                    qT_tiles[1] = qTp.tile([D, NH, 512], fp16, tag="qT",
                                           name="qT_1")
                    emit_proj(1, [0, 1], [])
                else:
                    emit_proj(j, range(NH + 1), range(4))

                # ---------------- attention -------------------------------
                finj = finp.tile([D, NH, 512], fp16, tag="fin",
                                 name=f"fin_{j}")
                fin_tiles[j] = finj
                nkb = 4 * j + 4
                nquad = nkb // 4
                # pace pending o_proj pops across this slice's quad
                # iterations so the last heads aren't starved of PE filler
                slots = NH * max(nquad - 1, 1)
                pop_frac = [0.0]
                pop_rate = len(pending_oproj) / slots if slots else 0.0
                for h in range(NH):
                    pso = pso_p.tile([D, 512], f32, tag="o")
                    se_l = {}

                    def score_quad(q, j=j, h=h):
                        pss = pss_p.tile([D, 4, 512], f32, tag="s")
                        se = sep.tile([D, 4, 512], fp16, tag="se")
                        if q == j:
                            # diagonal quad: blocks 0/1 exp'd over their
                            # trimmed ranges with a triangle mask; blocks
                            # 2/3 packed into one bank ([0:256]+[256:512])
                            # so one exp and one [tri|ones|zero|tri] mask
                            # cover both
                            for i in (0, 1):
                                kb = 4 * q + i
                                r0 = trim0[i]
                                nc.tensor.matmul(
                                    pss[:, i, r0:512],
                                    kT[:, 128 * kb:128 * (kb + 1)],
                                    qTj[:, h, r0:512],
                                    start=True, stop=True)
                                nc.scalar.activation(
                                    se[:, i, r0:512], pss[:, i, r0:512],
                                    Act.Exp,
                                    bias=ebias_t, scale=float(inv_sqrt_d))
                                m0 = 128 * i
                                nc.vector.tensor_mul(
                                    se[:, i, m0:m0 + 128],
                                    se[:, i, m0:m0 + 128],
                                    mask_t[:, 0:128])
                            nc.tensor.matmul(
                                pss[:, 2, 0:256],
                                kT[:, 128 * (4 * q + 2):128 * (4 * q + 3)],
                                qTj[:, h, 256:512], start=True, stop=True)
                            nc.tensor.matmul(
                                pss[:, 2, 256:512],
                                kT[:, 128 * (4 * q + 3):128 * (4 * q + 4)],
                                qTj[:, h, 256:512], start=True, stop=True)
                            nc.scalar.activation(
                                se[:, 2, :], pss[:, 2, :], Act.Exp,
                                bias=ebias_t, scale=float(inv_sqrt_d))
                            nc.vector.tensor_mul(se[:, 2, :], se[:, 2, :],
                                                 mask_t)
                        else:
                            for i in range(4):
                                kb = 4 * q + i
                                nc.tensor.matmul(
                                    pss[:, i, :],
                                    kT[:, 128 * kb:128 * (kb + 1)],
                                    qTj[:, h, :],
                                    start=True, stop=True)
                            # one exp covers all four blocks
                            nc.scalar.activation(
                                se, pss, Act.Exp,
                                bias=ebias_t, scale=float(inv_sqrt_d))
                        se_l[q] = se

                    score_quad(0)
                    if nquad == 1:
                        emit_norm()
                    for q in range(nquad):
                        if q + 1 < nquad:
                            score_quad(q + 1)
                            if q == 0:
                                # previous head's normalization: emitted one
                                # quad-exp deep so its Ln (waiting on the
                                # ssq matmul) never stalls the ACT queue
                                emit_norm()
                            pop_frac[0] += pop_rate
                            while pop_frac[0] >= 1.0:
                                pop_frac[0] -= 1.0
                                emit_oproj(1)
                        se = se_l.pop(q)
                        if q == j:
                            for i in (0, 1):
                                kb = 4 * q + i
                                r0 = trim0[i]
                                nc.tensor.matmul(pso[:, r0:512],
                                                 veff[:, kb, :],
                                                 se[:, i, r0:512],
                                                 start=(kb == 0), stop=False)
                            nc.tensor.matmul(pso[:, 256:512],
                                             veff[:, 4 * q + 2, :],
                                             se[:, 2, 0:256],
                                             start=False, stop=False)
                            nc.tensor.matmul(pso[:, 256:512],
                                             veff[:, 4 * q + 3, :],
                                             se[:, 2, 256:512],
                                             start=False, stop=True)
                        else:
                            for i in range(4):
                                kb = 4 * q + i
                                nc.tensor.matmul(pso, veff[:, kb, :],
                                                 se[:, i, :],
                                                 start=(kb == 0), stop=False)

                    # copy u out early so the psum slot frees without
                    # waiting on the normalization chain (the deferred norm
                    # emission hides this latency from the ACT queue)
                    u_f = ufp.tile([D, 512], f32, tag="uf")
                    if j <= 1:
                        # ACT is cool in the early slices; DVE is the hot
                        # engine there (RoPE + norm chains)
                        nc.scalar.copy(out=u_f, in_=pso)
                    else:
                        nc.vector.tensor_copy(u_f, pso)
                    sq = ufp.tile([D, 512], fp16, tag="sq")
                    nc.vector.tensor_mul(sq, u_f, u_f)
                    psss = pso_p.tile([D, 512], f32, tag="o", name="psss")
                    nc.tensor.matmul(psss, ones_t, sq, start=True, stop=True)

                    def norm(j=j, h=h, psss=psss, u_f=u_f, finj=finj):
                        lnr = ctmp.tile([D, 512], f32, tag="lnr")
                        nc.scalar.activation(lnr, psss, Act.Ln,
                                             scale=1.0 / 128.0)
                        comb = ctmp.tile([D, 512], f32, tag="comb")
                        nc.scalar.activation(comb, lnr, Act.Exp, scale=-0.5)
                        nc.vector.tensor_mul(finj[:, h, :], u_f, comb)

                    pending_norm[0] = norm
                    if j == NJ - 1 and h == NH - 1:
                        # last head: emit inline so the tail o_proj (which
                        # reads every head of finalT) starts sooner
                        emit_norm()

                # flush any o_proj tiles the pair loops didn't absorb
                emit_oproj(len(pending_oproj))

                # queue o_proj for this slice (interleaved into the next
                # slice's attention pair loops); the last slice is handled
                # by the wave-pipelined tail below
                for lb in range(4 * j, 4 * j + 4) if j < NJ - 1 else []:
                    for osl in range(HID // 512):
                        def oproj(j=j, lb=lb, osl=osl):
                            lrel = slice(128 * (lb - 4 * j),
                                         128 * (lb - 4 * j) + 128)
                            osl_s = slice(512 * osl, 512 * (osl + 1))
                            fin = fin_tiles[j]
                            ps = mm_p.tile([D, 512], f32, tag="mm",
                                           name="psop")
                            for h in range(NH):
                                nc.tensor.matmul(ps, fin[:, h, lrel],
                                                 wo_t[:, h, osl_s],
                                                 start=(h == 0),
                                                 stop=(h == NH - 1))
                            ob = dout.tile([D, 512], fp16, tag="ob")
                            # copy engine: during attn(1) DVE is the hot
                            # engine (RoPE) -> use ACT for o_proj(0); during
                            # attn(2,3) ACT is hot (exp) -> DVE; the tail
                            # flush of o_proj(3) alternates
                            if j == 0:
                                use_act = True
                            elif j == 3:
                                use_act = (lb + osl) % 2 == 1
                            else:
                                use_act = False
                            if use_act:
                                nc.scalar.copy(out=ob, in_=ps)
                            else:
                                nc.vector.tensor_copy(ob, ps)
                            nc.sync.dma_start(
                                out=part[128 * lb:128 * (lb + 1), osl_s],
                                in_=ob)
                        pending_oproj.append(oproj)

            # tail: last head's norm, then the last slice's o_proj in two
            # 8-tile waves across ALL psum banks: each wave accumulates
            # heads 0..2 first (ready before the last head's deferred norm
            # lands), then finishes with head 3 and drains
            emit_norm()
            finj3 = fin_tiles[NJ - 1]
            wave_sizes = (6, 4, 4, 2)
            wave_base = 0
            for wave, wsz in enumerate(wave_sizes):
                regions = [
                    mm_p.tile([D, 512], f32, tag="mm", name=f"tw{wave}a"),
                    mm_p.tile([D, 512], f32, tag="mm", name=f"tw{wave}b")]
                if wsz > 2:
                    pa = pss_p.tile([D, 4, 512], f32, tag="s",
                                    name=f"tw{wave}c")
                    regions += [pa[:, k, :] for k in range(4)]
                idxs = list(range(wave_base, wave_base + wsz))
                wave_base += wsz
                for m, idx in enumerate(idxs):
                    lrel = slice(128 * (idx // 4), 128 * (idx // 4) + 128)
                    osl_s = slice(512 * (idx % 4), 512 * (idx % 4) + 512)
                    for h in range(NH - 1):
                        nc.tensor.matmul(regions[m], finj3[:, h, lrel],
                                         wo_t[:, h, osl_s],
                                         start=(h == 0), stop=False,
                                         skip_group_check=True)
                for m, idx in enumerate(idxs):
                    lb = 4 * (NJ - 1) + idx // 4
                    lrel = slice(128 * (idx // 4), 128 * (idx // 4) + 128)
                    osl_s = slice(512 * (idx % 4), 512 * (idx % 4) + 512)
                    nc.tensor.matmul(regions[m], finj3[:, NH - 1, lrel],
                                     wo_t[:, NH - 1, osl_s],
                                     start=False, stop=True,
                                     skip_group_check=True)
                    ob = dout.tile([D, 512], fp16, tag="ob")
                    if m % 2 == 0:
                        nc.vector.tensor_copy(ob, regions[m])
                    else:
                        nc.scalar.copy(out=ob, in_=regions[m])
                    (nc.sync if m % 2 == 0 else nc.scalar).dma_start(
                        out=part[128 * lb:128 * (lb + 1), osl_s], in_=ob)

    nc.finalize()
    return nc


def _causal_masks():
    # [D, 512] = [tri | ones | zeros | tri] for the packed second-diagonal
    # pair (block ib=2 in cols [0:256] of the shared bank, ib=3 in
    # [256:512]); the in-block causal triangle (partition p = k offset,
    # col c = q offset: 1 iff q >= k) at cols [0:128] is reused for the
    # ib=0/1 single-block masks.
    m = np.zeros((D, 512), np.float32)
    for p in range(D):
        m[p, p:128] = 1.0
        m[p, 128:256] = 1.0
        m[p, 384 + p:512] = 1.0
    return m


def kernel(hidden_states, cos, sin, Wq, Wk, Wv, Wo,
           lambda_q1, lambda_k1, lambda_q2, lambda_k2, rms_weight):
    from concourse.bass_utils import run_bass_kernel_spmd

    fp16 = np.float16

    length = hidden_states.shape[1]
    if length not in _CACHE:
        _CACHE[length] = _build(length)
    nc = _CACHE[length]

    hidden_states = np.asarray(hidden_states, np.float32)
    cos = np.asarray(cos, np.float32)
    sin = np.asarray(sin, np.float32)

    lam_full = np.float32(
        np.exp(np.float32(np.dot(np.asarray(lambda_q1, np.float32),
                                 np.asarray(lambda_k1, np.float32)))
               + np.float32(np.dot(np.asarray(lambda_q2, np.float32),
                                   np.asarray(lambda_k2, np.float32))))
        + np.float32(LAMBDA_INIT))
    lam_arr = np.full((D, 1), -lam_full, np.float32)
    masks = _causal_masks().astype(fp16)

    Wq = np.asarray(Wq, np.float32)
    Wk = np.asarray(Wk, np.float32)
    Wv = np.asarray(Wv, np.float32)
    # fold per-head-dim RMSNorm weight and the (1 - lambda_init) factor into
    # Wo's rows (row index r of the per-core Wo slice has head-dim r % D)
    wo_scale = (np.asarray(rms_weight, np.float32)
                * np.float32(1.0 - LAMBDA_INIT))
    Wo = np.asarray(Wo, np.float32) * np.tile(wo_scale, H)[:, None]

    # sign-fold the rotate_half into the sin table: rows 0..63 negated
    sin_fold = np.concatenate([-sin[:, :, 0:64], sin[:, :, 64:128]], axis=2)

    in_maps = []
    for b in range(B):
        hsT_b = np.ascontiguousarray(hidden_states[b].T.astype(fp16))
        # cs[:, 0, :] = cos^T, cs[:, 1, :] = sign-folded sin^T
        cs_b = np.ascontiguousarray(np.stack(
            [cos[b].T, sin_fold[b].T], axis=1).astype(fp16))
        for t in range(4):
            vlo, vhi = t // 2, t // 2 + 2
            in_maps.append({
                "hsT": hsT_b,
                "cs": cs_b,
                "wqkv": np.ascontiguousarray(np.concatenate(
                    [Wq[:, 512 * t:512 * (t + 1)],
                     Wk[:, 128 * t:128 * (t + 1)],
                     Wv[:, 128 * vlo:128 * (vlo + 1)],
                     Wv[:, 128 * vhi:128 * (vhi + 1)]],
                    axis=1).astype(fp16)),
                "wo": np.ascontiguousarray(
                    Wo[512 * t:512 * (t + 1), :].astype(fp16)),
                "lam": lam_arr,
                "masks": masks,
            })

    trace = bool(os.environ.get("DIFFATTN_TRACE"))
    res = run_bass_kernel_spmd(nc, in_maps, list(range(8)), trace=trace)
    kernel.last_results = res

    out = np.empty((B, length, HID), np.float32)
    for b in range(B):
        acc = res.results[4 * b]["part"].astype(np.float32)
        for t in range(1, 4):
            acc = acc + res.results[4 * b + t]["part"].astype(np.float32)
        out[b] = acc
    return out
